# revision 13
# baseline (speedup 1.0000x reference)
"""Trainium2 Bass kernel for a dense decoder block (LN->MHA->res, LN->FFN->res).

Sharding (8 cores, one NEFF, SPMD-uniform addressing):
  - LN1 token-parallel (512-token chunk/core) -> AllGather of normalized acts
    quantized to fp8e4m3 (x32-scaled weights keep fp8 out of subnormals).
  - QKV + attention head-parallel (2 heads/core, causal, unstable softmax --
    exact because masked logits multiply to 0 post-exp).
  - AllToAll redistributes attention values (fp8): head-shards -> token-shards.
  - proj + residual + LN2 + FFN token-parallel with fp8 weights streamed.
  - LN affine params are folded into the following matmul weights on host.

All heavy GEMMs run in fp8e4m3 with MatmulPerfMode.DoubleRow (256-row
contraction per instruction, 0.5 cycles/output column -> 4x f32r MACs/cycle).
Weights are pre-scaled by 32 on host; PSUM drains apply 1/32 via the
activation-engine scale. Attention (scores/softmax/AV) runs in bf16.
LN statistics use the ones-matmul trick in f32r (1 cycle/row).
Activations stay channel-major [C, tokens]; v is produced token-major
directly by swapping matmul operands, so no transposes are needed.
"""

import math

import numpy as np
import ml_dtypes

import concourse.bass as bass
import concourse.mybir as mybir
import concourse.tile as tile
from concourse import bacc
from concourse import bass_utils

F32 = mybir.dt.float32
F32R = mybir.dt.float32r
BF16 = mybir.dt.bfloat16
F8 = mybir.dt.float8e4
AF = mybir.ActivationFunctionType
OP = mybir.AluOpType
PM = mybir.MatmulPerfMode

N_CORES = 8
B = 2
C = 2048
H = 16
HD = 128
F = 8192
NT = B * 2048                       # total tokens (B*T with T=2048)
H_PER_CORE = H // N_CORES           # 2
NCT = C // 128                      # 16 channel tiles
NB = C // 256                       # 8 DoubleRow contraction blocks
NFT = F // 128                      # 64 ffn tiles
NFB = F // 256                      # 32 ffn DoubleRow blocks
EPS = 1e-5
SCALE = 1.0 / math.sqrt(HD)
WS = 32.0                           # host-side weight scale (drains apply 1/WS)
RWS = 1.0 / WS
GELU = AF.Gelu_apprx_tanh


def r32(ap):
    return ap.bitcast(F32R)


def _ln_finish(nc, pool_small, ps_sum, ps_ssq, n_tok, ncols, tagpfx):
    """From broadcast sum/sumsq psums produce SBUF rstd/shift [128, ncols]."""
    mean = pool_small.tile([128, ncols], F32, tag=f"{tagpfx}_mean", name="mean")
    ex2 = pool_small.tile([128, ncols], F32, tag=f"{tagpfx}_ex2", name="ex2")
    nc.vector.tensor_scalar_mul(mean[:], ps_sum[:], 1.0 / n_tok)
    nc.vector.tensor_scalar_mul(ex2[:], ps_ssq[:], 1.0 / n_tok)
    msq = pool_small.tile([128, ncols], F32, tag=f"{tagpfx}_msq", name="msq")
    nc.vector.tensor_mul(msq[:], mean[:], mean[:])
    varp = pool_small.tile([128, ncols], F32, tag=f"{tagpfx}_varp", name="varp")
    nc.vector.scalar_tensor_tensor(varp[:], ex2[:], EPS, msq[:],
                                   op0=OP.add, op1=OP.subtract)
    std = pool_small.tile([128, ncols], F32, tag=f"{tagpfx}_std", name="std")
    nc.scalar.sqrt(std[:], varp[:])
    rstd_bc = pool_small.tile([128, ncols], F32, tag=f"{tagpfx}_rstd", name="rstd")
    nc.vector.reciprocal(rstd_bc[:], std[:])
    shift_bc = pool_small.tile([128, ncols], F32, tag=f"{tagpfx}_shift", name="shift")
    nc.vector.scalar_tensor_tensor(shift_bc[:], mean[:], -1.0, rstd_bc[:],
                                   op0=OP.mult, op1=OP.mult)
    return rstd_bc, shift_bc


def build_decoder(T=2048, collectives=True):
    """Build the SPMD decoder-block program for seq length T (2048 = real)."""
    NTOK = B * T
    CH = NTOK // N_CORES            # tokens per core chunk (512)
    NQS = max(1, T // 512)          # q slices of 512 per batch elem
    QS = min(512, T)
    NVT = NTOK // 128               # token-major v tiles (32)
    S_SUB = CH // 128               # 128-token subtiles per chunk (4)

    nc = bacc.Bacc("TRN2", target_bir_lowering=False, debug=False,
                   num_devices=N_CORES)

    # ---- I/O ----
    xt = nc.dram_tensor("xt", [C, CH], F32, kind="ExternalInput").ap()
    wq8 = nc.dram_tensor("wq8", [128, 2 * NB * 2, 128], F8, kind="ExternalInput").ap()
    wk8 = nc.dram_tensor("wk8", [128, 2 * NB * 2, 128], F8, kind="ExternalInput").ap()
    wv8 = nc.dram_tensor("wv8", [128, NB * 2, 256], F8, kind="ExternalInput").ap()
    bq = nc.dram_tensor("bq", [256, 1], F32, kind="ExternalInput").ap()
    bk = nc.dram_tensor("bk", [256, 1], F32, kind="ExternalInput").ap()
    bv_bc = nc.dram_tensor("bv_bc", [128, 256], F32, kind="ExternalInput").ap()
    wproj8 = nc.dram_tensor("wproj8", [128, NCT * NB * 2, 128], F8,
                            kind="ExternalInput").ap()
    bproj = nc.dram_tensor("bproj", [C, 1], F32, kind="ExternalInput").ap()
    wf18 = nc.dram_tensor("wf18", [NFT, 128, NB * 2, 128], F8,
                          kind="ExternalInput").ap()
    bf1 = nc.dram_tensor("bf1", [F, 1], F32, kind="ExternalInput").ap()
    wf28 = nc.dram_tensor("wf28", [NCT, 128, NFB * 2, 128], F8,
                          kind="ExternalInput").ap()
    bf2 = nc.dram_tensor("bf2", [C, 1], F32, kind="ExternalInput").ap()
    masks = nc.dram_tensor("masks", [128, 4, QS], BF16, kind="ExternalInput").ap()
    out = nc.dram_tensor("out", [C, CH], F32, kind="ExternalOutput").ap()

    RG = [list(range(N_CORES))]

    with tile.TileContext(nc) as tc:
        with tc.tile_pool(name="dram", bufs=1, space="DRAM") as dram, \
             tc.tile_pool(name="persist", bufs=1) as persist:
            n1_bounce = dram.tile([C, CH], F8, tag="n1_bounce", name="n1_bounce")
            n1_full = dram.tile([N_CORES * C, CH], F8, tag="n1_full",
                                name="n1_full", addr_space="Shared")
            a2a_in = [dram.tile([C // 2, CH], F8, tag=f"a2a_in{h}",
                                name="a2a_in") for h in range(2)]
            a2a_out = [dram.tile([C // 2, CH], F8, tag=f"a2a_out{h}",
                                 name="a2a_out") for h in range(2)]

            # x tiles are the critical path at t=0: issue their DMAs first.
            xt_view = xt.rearrange("(k p) t -> p k t", p=128)
            ones_sq = persist.tile([128, 128], F32, tag="ones_sq", name="ones_sq")
            ones_bf = persist.tile([128, 128], BF16, tag="ones_bf", name="ones_bf")
            nc.vector.memset(ones_sq[:], 1.0)
            nc.vector.tensor_copy(ones_bf[:], ones_sq[:])
            masks_sb = persist.tile([128, 4, QS], BF16, tag="masks", name="masks_sb")
            nc.sync.dma_start(masks_sb[:], masks)
            bq_sb = persist.tile([128, 2, 1], F32, tag="bq", name="bq_sb")
            bk_sb = persist.tile([128, 2, 1], F32, tag="bk", name="bk_sb")
            nc.sync.dma_start(bq_sb[:], bq.rearrange("(o p) u -> p o u", p=128))
            nc.sync.dma_start(bk_sb[:], bk.rearrange("(o p) u -> p o u", p=128))
            bv_sb = persist.tile([128, 256], F32, tag="bv", name="bv_sb")
            nc.sync.dma_start(bv_sb[:], bv_bc)
            bproj_sb = persist.tile([128, NCT, 1], F32, tag="bproj", name="bproj_sb")
            nc.sync.dma_start(bproj_sb[:], bproj.rearrange("(o p) u -> p o u", p=128))
            bf1_sb = persist.tile([128, NFT, 1], F32, tag="bf1", name="bf1_sb")
            nc.sync.dma_start(bf1_sb[:], bf1.rearrange("(o p) u -> p o u", p=128))
            bf2_sb = persist.tile([128, NCT, 1], F32, tag="bf2", name="bf2_sb")
            nc.sync.dma_start(bf2_sb[:], bf2.rearrange("(o p) u -> p o u", p=128))

            # r1 survives proj -> final residual add; x survives LN1 -> proj.
            r1_sb = persist.tile([128, NCT, CH], F32, tag="r1", name="r1_sb")

            with tc.tile_pool(name="xpool", bufs=1) as xpool:
                x_sb = xpool.tile([128, NCT, CH], F32, tag="x_sb", name="x_sb")
                n2pool = tc.alloc_tile_pool(name="n2pool", bufs=1)
                n2_sb = n2pool.tile([128, NCT, CH], F8, tag="n2_sb",
                                    name="n2_sb")
                projw = tc.alloc_tile_pool(name="projw", bufs=1)
                wp_sb = projw.tile([128, NCT * NB * 2, 128], F8, tag="wp",
                                   name="wp_sb")
                wqkvp = tc.alloc_tile_pool(name="wqkv", bufs=1)
                wq_sb = wqkvp.tile([128, 2 * NB * 2, 128], F8, tag="wq",
                                   name="wq_sb")
                wk_sb = wqkvp.tile([128, 2 * NB * 2, 128], F8, tag="wk",
                                   name="wk_sb")
                wv_sb = wqkvp.tile([128, NB * 2, 256], F8, tag="wv",
                                   name="wv_sb")
                xbfpool = tc.alloc_tile_pool(name="xbfpool", bufs=1)
                x_bf = xbfpool.tile([128, NCT, CH], BF16, tag="x_bf", name="x_bf")

                # ================= Phase A: LN1 on own chunk =================
                with tc.tile_pool(name="lnA", bufs=3) as lnA, \
                     tc.tile_pool(name="lnA_small", bufs=1) as lnAs, \
                     tc.tile_pool(name="n1pool", bufs=4) as n1pool, \
                     tc.tile_pool(name="psA", bufs=1, space="PSUM") as psA:
                    ps_sum = psA.tile([128, CH], F32, tag="sum", name="ps_sum")
                    ps_ssq = psA.tile([128, CH], F32, tag="ssq", name="ps_ssq")
                    # quarter-granularity loads: tile-level WAR tracking would
                    # serialize 16 per-slice DMAs against their readers
                    for q in range(4):
                        nc.sync.dma_start(x_sb[:, 4 * q:4 * (q + 1), :],
                                          xt_view[:, 4 * q:4 * (q + 1), :])
                    for k in range(NCT):
                        nc.scalar.activation(x_bf[:, k, :], x_sb[:, k, :],
                                             AF.Identity)
                        sq = lnA.tile([128, CH], BF16, tag="sq", name="sq")
                        nc.vector.tensor_mul(sq[:], x_bf[:, k, :], x_bf[:, k, :])
                        nc.tensor.matmul(ps_sum[:], ones_bf[:], x_bf[:, k, :],
                                         start=(k == 0), stop=(k == NCT - 1))
                        nc.tensor.matmul(ps_ssq[:], ones_bf[:], sq[:],
                                         start=(k == 0), stop=(k == NCT - 1))
                    # queue the QKV weight loads behind the x tiles, ahead of
                    # the dep-gated bounce writes (DMA queue is FIFO)
                    nc.sync.dma_start(wq_sb[:], wq8)
                    nc.sync.dma_start(wk_sb[:], wk8)
                    nc.sync.dma_start(wv_sb[:], wv8)
                    rstd_bc, shift_bc = _ln_finish(nc, lnAs, ps_sum, ps_ssq,
                                                   C, CH, "ln1")
                    rstd_bf = lnAs.tile([128, CH], BF16, tag="ln1_rstdbf",
                                        name="rstd_bf")
                    nc.scalar.activation(rstd_bf[:], rstd_bc[:], AF.Identity)
                    shift_bf = lnAs.tile([128, CH], BF16, tag="ln1_shiftbf",
                                         name="shift_bf")
                    nc.scalar.activation(shift_bf[:], shift_bc[:], AF.Identity)
                    n1_view = n1_bounce[:].rearrange("(k p) t -> p k t", p=128)
                    for k in range(NCT):
                        # per-tile staging tiles (bufs=4) avoid WAR serialization
                        # against the bounce DMA reads; Pool takes a few tiles
                        # (its tensor ops cost ~2.6x DVE's).
                        if k % 4 == 3:
                            tmp = lnA.tile([128, CH], BF16, tag="apP",
                                           name="tmp")
                            n1t = n1pool.tile([128, CH], F8, tag="n1P",
                                              name="n1t")
                            nc.gpsimd.tensor_mul(tmp[:], x_bf[:, k, :], rstd_bf[:])
                            nc.gpsimd.tensor_add(n1t[:], tmp[:], shift_bf[:])
                        else:
                            tmp = lnA.tile([128, CH], BF16, tag="apV",
                                           name="tmp")
                            n1t = n1pool.tile([128, CH], F8, tag="n1V",
                                              name="n1t")
                            nc.vector.tensor_mul(tmp[:], x_bf[:, k, :], rstd_bf[:])
                            nc.vector.tensor_add(n1t[:], tmp[:], shift_bf[:])
                        nc.sync.dma_start(n1_view[:, k, :], n1t[:])
                xbfpool.release()

                if collectives:
                    nc.gpsimd.collective_compute(
                        "AllGather", OP.bypass, replica_groups=RG,
                        ins=[n1_bounce.opt()], outs=[n1_full.opt()])
                else:  # timing variant: plain copy keeps the dependency edge
                    nc.sync.dma_start(n1_full[0:C, :], n1_bounce[:])

                # ====== Phase B: QKV (all tokens, own 2 heads, fp8 DR) ======
                with tc.tile_pool(name="qkv_sb", bufs=1) as qkvp:
                    q_sb = qkvp.tile([128, 2, NTOK], BF16, tag="q_sb", name="q_sb")
                    k_sb = qkvp.tile([128, 2, NTOK], BF16, tag="k_sb", name="k_sb")
                    v_sb = qkvp.tile([128, NVT, 256], BF16, tag="v_sb", name="v_sb")

                    with tc.tile_pool(name="n1t", bufs=3) as n1tp, \
                         tc.tile_pool(name="psQK", bufs=1, space="PSUM") as psQK, \
                         tc.tile_pool(name="psV", bufs=1, space="PSUM") as psV:
                        nf_view = n1_full[:].rearrange("(r k p) t -> r p k t",
                                                       r=N_CORES, p=128)
                        for r in range(N_CORES):
                            n1c = n1tp.tile([128, NCT, CH], F8, tag="n1c",
                                            name="n1c")
                            nc.sync.dma_start(n1c[:], nf_view[r])
                            ps_q = [psQK.tile([128, CH], F32, tag=f"q{o}",
                                              name=f"ps_q{o}") for o in range(2)]
                            ps_k = [psQK.tile([128, CH], F32, tag=f"k{o}",
                                              name=f"ps_k{o}") for o in range(2)]
                            ps_v = [psV.tile([128, 256], F32, tag=f"v{s}",
                                             name=f"ps_v{s}") for s in range(S_SUB)]
                            for b in range(NB):
                                rhs = n1c[:, 2 * b:2 * b + 2, :]
                                st, sp = (b == 0), (b == NB - 1)
                                for o in range(2):
                                    nc.tensor.matmul(
                                        ps_q[o][:],
                                        wq_sb[:, (o * NB + b) * 2:(o * NB + b) * 2 + 2, :],
                                        rhs, start=st, stop=sp, perf_mode=PM.DoubleRow)
                                    nc.tensor.matmul(
                                        ps_k[o][:],
                                        wk_sb[:, (o * NB + b) * 2:(o * NB + b) * 2 + 2, :],
                                        rhs, start=st, stop=sp, perf_mode=PM.DoubleRow)
                                for s in range(S_SUB):
                                    nc.tensor.matmul(
                                        ps_v[s][:],
                                        n1c[:, 2 * b:2 * b + 2, 128 * s:128 * (s + 1)],
                                        wv_sb[:, 2 * b:2 * b + 2, :],
                                        start=st, stop=sp, perf_mode=PM.DoubleRow)
                            for o in range(2):
                                nc.vector.tensor_scalar(
                                    q_sb[:, o, CH * r:CH * (r + 1)], ps_q[o][:],
                                    RWS, bq_sb[:, o, :], op0=OP.mult, op1=OP.add)
                                nc.vector.tensor_scalar(
                                    k_sb[:, o, CH * r:CH * (r + 1)], ps_k[o][:],
                                    RWS, bk_sb[:, o, :], op0=OP.mult, op1=OP.add)
                            for s in range(S_SUB):
                                nc.vector.scalar_tensor_tensor(
                                    v_sb[:, r * S_SUB + s, :], ps_v[s][:], RWS,
                                    bv_sb[:], op0=OP.mult, op1=OP.add)

                    # ========= Phase B2: attention per (head, batch) =========
                    WPC = NCT * NB * 2 // 8
                    for pc in range(8):
                        nc.sync.dma_start(wp_sb[:, WPC * pc:WPC * (pc + 1), :],
                                          wproj8[:, WPC * pc:WPC * (pc + 1), :])
                    with tc.tile_pool(name="attn_e", bufs=4) as ep, \
                         tc.tile_pool(name="attn_acc", bufs=2) as accp, \
                         tc.tile_pool(name="attn_small", bufs=3) as asml, \
                         tc.tile_pool(name="vals", bufs=3) as valsp, \
                         tc.tile_pool(name="psS", bufs=2, space="PSUM") as psS, \
                         tc.tile_pool(name="psAV", bufs=2, space="PSUM") as psAV, \
                         tc.tile_pool(name="psDen", bufs=2, space="PSUM") as psDen:
                        for h in range(H_PER_CORE):
                            for bb in range(B):
                                for j in range(NQS):
                                    ni = 4 * (j + 1) if QS == 512 else T // 128
                                    ps_av = psAV.tile([128, QS], F32, tag="av",
                                                      name="ps_av")
                                    ps_den = psDen.tile([128, QS], F32, tag="den",
                                                        name="ps_den")
                                    e_acc = accp.tile([128, QS], BF16, tag="eacc",
                                                      name="e_acc")
                                    qtok = bb * T + j * QS
                                    for u in range(ni // 2):
                                        # paired score tiles share one Exp call
                                        # over [128, 1024] (amortizes Act setup)
                                        ps_s2 = psS.tile([128, 2, QS], F32,
                                                         tag="s2", name="ps_s2")
                                        for hf in range(2):
                                            i = 2 * u + hf
                                            ktok = bb * T + i * 128
                                            nc.tensor.matmul(
                                                ps_s2[:, hf, :],
                                                k_sb[:, h, ktok:ktok + 128],
                                                q_sb[:, h, qtok:qtok + QS],
                                                start=True, stop=True)
                                        e2 = ep.tile([128, 2, QS], BF16, tag="e2",
                                                     name="e2")
                                        nc.scalar.activation(e2[:], ps_s2[:],
                                                             AF.Exp, bias=0.0,
                                                             scale=SCALE)
                                        d0 = 2 * u - (ni - 4)
                                        if d0 >= 0:
                                            nc.vector.tensor_mul(
                                                e2[:], e2[:],
                                                masks_sb[:, d0:d0 + 2, :])
                                        for hf in range(2):
                                            i = 2 * u + hf
                                            # hybrid denominator: diagonal tiles
                                            # accumulate on PE, the rest on DVE
                                            if i < ni - 4:
                                                if i == 0:
                                                    nc.vector.tensor_copy(
                                                        e_acc[:], e2[:, hf, :])
                                                else:
                                                    nc.vector.tensor_add(
                                                        e_acc[:], e_acc[:],
                                                        e2[:, hf, :])
                                            else:
                                                nc.tensor.matmul(
                                                    ps_den[:], ones_bf[:],
                                                    e2[:, hf, :],
                                                    start=(i == ni - 4),
                                                    stop=(i == ni - 1 and ni == 4))
                                            tt = (bb * T + i * 128) // 128
                                            nc.tensor.matmul(
                                                ps_av[:],
                                                v_sb[:, tt, 128 * h:128 * (h + 1)],
                                                e2[:, hf, :],
                                                start=(i == 0), stop=(i == ni - 1))
                                    if ni > 4:
                                        nc.tensor.matmul(ps_den[:], ones_bf[:],
                                                         e_acc[:], start=False,
                                                         stop=True)
                                    rec_bc = asml.tile([128, QS], F32, tag="rec",
                                                       name="rec_bc")
                                    nc.vector.reciprocal(rec_bc[:], ps_den[:])
                                    vtile = valsp.tile([128, QS], F8, tag="vt",
                                                       name="vtile")
                                    nc.vector.tensor_mul(vtile[:], ps_av[:],
                                                         rec_bc[:])
                                    ncol0 = bb * T + j * QS
                                    for part in range(max(1, QS // CH)):
                                        jg = (ncol0 + part * CH) // CH
                                        w = min(CH, QS)
                                        nc.sync.dma_start(
                                            a2a_in[h][128 * jg:128 * (jg + 1), :],
                                            vtile[:, part * w:(part + 1) * w])
                            if h == 0:
                                # h=0 values complete at half-time: overlap the
                                # first AllToAll with the h=1 attention pass
                                if collectives:
                                    nc.gpsimd.collective_compute(
                                        "AllToAll", OP.bypass, replica_groups=RG,
                                        ins=[a2a_in[0].opt()],
                                        outs=[a2a_out[0].opt()])
                                else:
                                    nc.sync.dma_start(a2a_out[0][:], a2a_in[0][:])

                wqkvp.release()
                if collectives:
                    nc.gpsimd.collective_compute(
                        "AllToAll", OP.bypass, replica_groups=RG,
                        ins=[a2a_in[1].opt()], outs=[a2a_out[1].opt()])
                else:
                    nc.sync.dma_start(a2a_out[1][:], a2a_in[1][:])

                # ====== Phase C: proj + residual + LN2 stats (own chunk) ======
                with tc.tile_pool(name="vf", bufs=1) as vfp, \
                     tc.tile_pool(name="pdrain", bufs=3) as pdp, \
                     tc.tile_pool(name="lnC_small", bufs=1) as lnCs, \
                     tc.tile_pool(name="psP", bufs=3, space="PSUM") as psP, \
                     tc.tile_pool(name="psP2", bufs=1, space="PSUM") as psP2:
                    vf_sb = vfp.tile([128, NB, 2, CH], F8, tag="vf",
                                     name="vf_sb")
                    for hs in range(2):
                        nc.sync.dma_start(
                            vf_sb[:, :, hs, :],
                            a2a_out[hs][:].rearrange("(r p) t -> p r t", p=128))
                    ps_sum2 = psP2.tile([128, CH], F32, tag="sum2", name="ps_sum2")
                    ps_ssq2 = psP2.tile([128, CH], F32, tag="ssq2", name="ps_ssq2")
                    for ot in range(NCT):
                        ps_p = psP.tile([128, CH], F32, tag="p", name="ps_p")
                        for b in range(NB):
                            nc.tensor.matmul(
                                ps_p[:],
                                wp_sb[:, (ot * NB + b) * 2:(ot * NB + b) * 2 + 2, :],
                                vf_sb[:, b, :, :],
                                start=(b == 0), stop=(b == NB - 1),
                                perf_mode=PM.DoubleRow)
                        p_t = pdp.tile([128, CH], F32, tag="p_t", name="p_t")
                        nc.scalar.activation(p_t[:], ps_p[:], AF.Identity,
                                             bias=bproj_sb[:, ot, :], scale=RWS)
                        nc.vector.tensor_add(r1_sb[:, ot, :], p_t[:],
                                             x_sb[:, ot, :])
                    # LN2 stats decoupled from the proj loop so PE is not
                    # latency-bound on each ot's drain chain
                    r1bfp = tc.alloc_tile_pool(name="r1bf", bufs=1)
                    r1_bf = r1bfp.tile([128, NCT, CH], BF16, tag="r1_bf",
                                       name="r1_bf")
                    for ot in range(NCT):
                        if ot % 2 == 0:
                            nc.scalar.activation(r1_bf[:, ot, :], r1_sb[:, ot, :],
                                                 AF.Identity)
                        else:
                            nc.vector.tensor_copy(r1_bf[:, ot, :],
                                                  r1_sb[:, ot, :])
                        sq2 = pdp.tile([128, CH], BF16, tag="sq2", name="sq2")
                        nc.vector.tensor_mul(sq2[:], r1_bf[:, ot, :],
                                             r1_bf[:, ot, :])
                        nc.tensor.matmul(ps_sum2[:], ones_bf[:], r1_bf[:, ot, :],
                                         start=(ot == 0), stop=(ot == NCT - 1))
                        nc.tensor.matmul(ps_ssq2[:], ones_bf[:], sq2[:],
                                         start=(ot == 0), stop=(ot == NCT - 1))
                    rstd2_bc, shift2_bc = _ln_finish(nc, lnCs, ps_sum2, ps_ssq2,
                                                     C, CH, "ln2")
                    rstd2_bf = lnCs.tile([128, CH], BF16, tag="ln2_rstdbf",
                                         name="rstd2_bf")
                    nc.scalar.activation(rstd2_bf[:], rstd2_bc[:], AF.Identity)
                    shift2_bf = lnCs.tile([128, CH], BF16, tag="ln2_shiftbf",
                                          name="shift2_bf")
                    nc.scalar.activation(shift2_bf[:], shift2_bc[:], AF.Identity)
                    for k in range(NCT):
                        if k % 4 == 3:
                            tmp2 = pdp.tile([128, CH], BF16, tag="ap2P",
                                            name="tmp2")
                            nc.gpsimd.tensor_mul(tmp2[:], r1_bf[:, k, :],
                                                 rstd2_bf[:])
                            nc.gpsimd.tensor_add(n2_sb[:, k, :], tmp2[:],
                                                 shift2_bf[:])
                        else:
                            tmp2 = pdp.tile([128, CH], BF16, tag="ap2V",
                                            name="tmp2")
                            nc.vector.tensor_mul(tmp2[:], r1_bf[:, k, :],
                                                 rstd2_bf[:])
                            nc.vector.tensor_add(n2_sb[:, k, :], tmp2[:],
                                                 shift2_bf[:])
                    r1bfp.release()

                projw.release()
                # =============== Phase D: FFN1 (fp8 DR) ===============
                with tc.tile_pool(name="hpool", bufs=1) as hpool, \
                     tc.tile_pool(name="w1", bufs=4) as w1p, \
                     tc.tile_pool(name="psH", bufs=3, space="PSUM") as psH:
                    h_sb = hpool.tile([128, NFT, CH], F8, tag="h_sb",
                                      name="h_sb")
                    for ft in range(NFT):
                        w1t = w1p.tile([128, NB * 2, 128], F8, tag="w1",
                                       name="w1t")
                        nc.sync.dma_start(w1t[:], wf18[ft])
                        ps_h = psH.tile([128, CH], F32, tag="h", name="ps_h")
                        for b in range(NB):
                            nc.tensor.matmul(
                                ps_h[:], w1t[:, 2 * b:2 * b + 2, :],
                                n2_sb[:, 2 * b:2 * b + 2, :],
                                start=(b == 0), stop=(b == NB - 1),
                                perf_mode=PM.DoubleRow)
                        nc.scalar.activation(h_sb[:, ft, :], ps_h[:], GELU,
                                             bias=bf1_sb[:, ft, :], scale=RWS)

                    # ============= Phase D2: FFN2 (fp8 DR) =============
                    with tc.tile_pool(name="w2", bufs=3) as w2p, \
                         tc.tile_pool(name="outp", bufs=3) as outp, \
                         tc.tile_pool(name="psF", bufs=2, space="PSUM") as psF:
                        for ot in range(NCT):
                            w2t = w2p.tile([128, NFB * 2, 128], F8, tag="w2",
                                           name="w2t")
                            nc.sync.dma_start(w2t[:], wf28[ot])
                            ps_f = psF.tile([128, CH], F32, tag="f",
                                            name="ps_f")
                            for g in range(NFB):
                                nc.tensor.matmul(
                                    ps_f[:], w2t[:, 2 * g:2 * g + 2, :],
                                    h_sb[:, 2 * g:2 * g + 2, :],
                                    start=(g == 0), stop=(g == NFB - 1),
                                    perf_mode=PM.DoubleRow)
                            f_t = outp.tile([128, CH], F32, tag="f_t",
                                            name="f_t")
                            nc.scalar.activation(f_t[:], ps_f[:], AF.Identity,
                                                 bias=bf2_sb[:, ot, :],
                                                 scale=RWS)
                            o_t = outp.tile([128, CH], F32, tag="o_t",
                                            name="o_t")
                            nc.vector.tensor_add(o_t[:], f_t[:],
                                                 r1_sb[:, ot, :])
                            nc.sync.dma_start(out[128 * ot:128 * (ot + 1), :],
                                              o_t[:])
                n2pool.release()

    nc.compile()
    return nc


# ----------------------------------------------------------------------------
# Host side
# ----------------------------------------------------------------------------

_NC_CACHE = {}


def _get_nc(T=2048):
    if T not in _NC_CACHE:
        _NC_CACHE[T] = build_decoder(T)
    return _NC_CACHE[T]


def _q8(a):
    """Quantize f32 -> fp8 e4m3 bytes with the x32 pre-scale."""
    return (np.ascontiguousarray(a, np.float32) * WS).astype(
        ml_dtypes.float8_e4m3).view(np.uint8)


def _dr_lhsT_flat(W):
    """[K, M] f32 -> DoubleRow lhsT SBUF layout [128, (M/128 * K/256 * 2), 128]."""
    K, M = W.shape
    a = W.reshape(K // 256, 2, 128, M // 128, 128).transpose(2, 3, 0, 1, 4)
    return np.ascontiguousarray(a.reshape(128, (M // 128) * (K // 256) * 2, 128))


def _dr_lhsT_tiles(W):
    """[K, M] f32 -> per-out-tile DoubleRow layout [M/128, 128, K/256 * 2, 128]."""
    K, M = W.shape
    a = W.reshape(K // 256, 2, 128, M // 128, 128).transpose(3, 2, 0, 1, 4)
    return np.ascontiguousarray(a.reshape(M // 128, 128, (K // 256) * 2, 128))


def _bf16(a):
    return np.ascontiguousarray(a).astype(ml_dtypes.bfloat16).view(np.uint16)


def _prep_inputs(x, Wqkv, bqkv, Wproj, bproj, Wf1, bf1, Wf2, bf2,
                 g1, b1, g2, b2):
    """Fold LN affines, slice heads per core, build per-core in_maps."""
    f32 = np.float32
    x = np.asarray(x, f32)
    Bx, T, Cx = x.shape
    NTOK = Bx * T
    CH = NTOK // N_CORES
    Wqkv = np.asarray(Wqkv, f32)
    bqkv = np.asarray(bqkv, f32)
    g1 = np.asarray(g1, f32); b1 = np.asarray(b1, f32)
    g2 = np.asarray(g2, f32); b2 = np.asarray(b2, f32)
    Wqkv_eff = g1[:, None] * Wqkv
    bqkv_eff = b1 @ Wqkv + bqkv
    Wf1 = np.asarray(Wf1, f32)
    bf1v = np.asarray(bf1, f32)
    Wf1_eff = g2[:, None] * Wf1
    bf1_eff = b2 @ Wf1 + bf1v
    Wproj = np.asarray(Wproj, f32)
    bprojv = np.asarray(bproj, f32)
    Wf2 = np.asarray(Wf2, f32)
    bf2v = np.asarray(bf2, f32)

    xt = np.ascontiguousarray(x.reshape(NTOK, Cx).T)        # [C, NT]

    QS = min(512, T)
    masks = np.zeros((128, 4, QS), f32)
    p = np.arange(128)[:, None]
    fcol = np.arange(QS)[None, :]
    for m in range(4):
        masks[:, m, :] = (p <= fcol - 128 * m).astype(f32)

    shared = {
        "wproj8": _q8(_dr_lhsT_flat(Wproj)),
        "bproj": bprojv.reshape(Cx, 1),
        "wf18": _q8(_dr_lhsT_tiles(Wf1_eff)),
        "bf1": bf1_eff.reshape(F, 1),
        "wf28": _q8(_dr_lhsT_tiles(Wf2)),
        "bf2": bf2v.reshape(Cx, 1),
        "masks": _bf16(masks),
    }
    in_maps = []
    for c in range(N_CORES):
        h0, h1 = 2 * c, 2 * c + 1
        qcols = np.concatenate([h0 * 384 + np.arange(128),
                                h1 * 384 + np.arange(128)])
        kcols = qcols + 128
        vcols = qcols + 256
        m = dict(shared)
        m["xt"] = np.ascontiguousarray(xt[:, c * CH:(c + 1) * CH])
        m["wq8"] = _q8(_dr_lhsT_flat(Wqkv_eff[:, qcols]))
        m["wk8"] = _q8(_dr_lhsT_flat(Wqkv_eff[:, kcols]))
        # v rhs SBUF layout: [128, K/256 * 2, 256]
        wv = Wqkv_eff[:, vcols].reshape(8, 2, 128, 256).transpose(2, 0, 1, 3)
        m["wv8"] = _q8(np.ascontiguousarray(wv.reshape(128, 16, 256)))
        m["bq"] = np.ascontiguousarray(bqkv_eff[qcols].reshape(256, 1))
        m["bk"] = np.ascontiguousarray(bqkv_eff[kcols].reshape(256, 1))
        m["bv_bc"] = np.ascontiguousarray(
            np.broadcast_to(bqkv_eff[vcols][None, :], (128, 256)))
        in_maps.append(m)
    return in_maps, (Bx, T, Cx, CH)


def kernel(x, Wqkv, bqkv, Wproj, bproj, Wf1, bf1, Wf2, bf2,
           g1, b1, g2, b2, _trace=False):
    in_maps, (Bx, T, Cx, CH) = _prep_inputs(
        x, Wqkv, bqkv, Wproj, bproj, Wf1, bf1, Wf2, bf2, g1, b1, g2, b2)
    nc = _get_nc(T)
    res = bass_utils.run_bass_kernel_spmd(
        nc, in_maps, core_ids=list(range(N_CORES)), trace=_trace)
    kernel.last_results = res
    NTOK = Bx * T
    out_t = np.empty((NTOK, Cx), np.float32)
    for c in range(N_CORES):
        out_t[c * CH:(c + 1) * CH, :] = res.results[c]["out"].T
    return out_t.reshape(Bx, T, Cx)


# revision 16
# speedup vs baseline: 1.0435x; 1.0435x over previous
"""Trainium2 Bass kernel for a dense decoder block (LN->MHA->res, LN->FFN->res).

Sharding (8 cores, one NEFF, SPMD-uniform addressing):
  - LN1 token-parallel (512-token chunk/core) -> AllGather of normalized acts
    quantized to fp8e4m3 (x32-scaled weights keep fp8 out of subnormals).
  - QKV + attention head-parallel (2 heads/core, causal, unstable softmax --
    exact because masked logits multiply to 0 post-exp).
  - AllToAll redistributes attention values (fp8): head-shards -> token-shards.
  - proj + residual + LN2 + FFN token-parallel with fp8 weights streamed.
  - LN affine params are folded into the following matmul weights on host.

All heavy GEMMs run in fp8e4m3 with MatmulPerfMode.DoubleRow (256-row
contraction per instruction, 0.5 cycles/output column -> 4x f32r MACs/cycle).
Weights are pre-scaled by 32 on host; PSUM drains apply 1/32 via the
activation-engine scale. Attention (scores/softmax/AV) runs in bf16.
LN statistics use the ones-matmul trick in f32r (1 cycle/row).
Activations stay channel-major [C, tokens]; v is produced token-major
directly by swapping matmul operands, so no transposes are needed.
"""

import math

import numpy as np
import ml_dtypes

import concourse.bass as bass
import concourse.mybir as mybir
import concourse.tile as tile
from concourse import bacc
from concourse import bass_utils

F32 = mybir.dt.float32
F32R = mybir.dt.float32r
BF16 = mybir.dt.bfloat16
F8 = mybir.dt.float8e4
AF = mybir.ActivationFunctionType
OP = mybir.AluOpType
PM = mybir.MatmulPerfMode

N_CORES = 8
B = 2
C = 2048
H = 16
HD = 128
F = 8192
NT = B * 2048                       # total tokens (B*T with T=2048)
H_PER_CORE = H // N_CORES           # 2
NCT = C // 128                      # 16 channel tiles
NB = C // 256                       # 8 DoubleRow contraction blocks
NFT = F // 128                      # 64 ffn tiles
NFB = F // 256                      # 32 ffn DoubleRow blocks
EPS = 1e-5
SCALE = 1.0 / math.sqrt(HD)
WS = 32.0                           # host-side weight scale (drains apply 1/WS)
RWS = 1.0 / WS
GELU = AF.Gelu_apprx_tanh


def r32(ap):
    return ap.bitcast(F32R)


def _ln_finish(nc, pool_small, ps_sum, ps_ssq, n_tok, ncols, tagpfx, lp=None):
    """From broadcast sum/sumsq psums produce SBUF rstd/shift [128, ncols].

    With lp set, outputs are bf16 (feeding the bf16 apply path)."""
    odt = BF16 if lp is not None else F32
    mean = pool_small.tile([128, ncols], F32, tag=f"{tagpfx}_mean", name="mean")
    ex2 = pool_small.tile([128, ncols], F32, tag=f"{tagpfx}_ex2", name="ex2")
    nc.vector.tensor_scalar_mul(mean[:], ps_sum[:], 1.0 / n_tok)
    nc.vector.tensor_scalar_mul(ex2[:], ps_ssq[:], 1.0 / n_tok)
    msq = pool_small.tile([128, ncols], F32, tag=f"{tagpfx}_msq", name="msq")
    nc.vector.tensor_mul(msq[:], mean[:], mean[:])
    varp = pool_small.tile([128, ncols], F32, tag=f"{tagpfx}_varp", name="varp")
    nc.vector.scalar_tensor_tensor(varp[:], ex2[:], EPS, msq[:],
                                   op0=OP.add, op1=OP.subtract)
    std = pool_small.tile([128, ncols], F32, tag=f"{tagpfx}_std", name="std")
    nc.scalar.sqrt(std[:], varp[:])
    rstd_bc = pool_small.tile([128, ncols], odt, tag=f"{tagpfx}_rstd", name="rstd")
    if lp is not None:
        with lp.allow_low_precision(reason="rstd broadcast feeds fp8 path"):
            nc.vector.reciprocal(rstd_bc[:], std[:])
    else:
        nc.vector.reciprocal(rstd_bc[:], std[:])
    shift_bc = pool_small.tile([128, ncols], odt, tag=f"{tagpfx}_shift", name="shift")
    nc.vector.scalar_tensor_tensor(shift_bc[:], mean[:], -1.0, rstd_bc[:],
                                   op0=OP.mult, op1=OP.mult)
    return rstd_bc, shift_bc


def build_decoder(T=2048, collectives=True):
    """Build the SPMD decoder-block program for seq length T (2048 = real)."""
    NTOK = B * T
    CH = NTOK // N_CORES            # tokens per core chunk (512)
    NQS = max(1, T // 512)          # q slices of 512 per batch elem
    QS = min(512, T)
    NVT = NTOK // 128               # token-major v tiles (32)
    S_SUB = CH // 128               # 128-token subtiles per chunk (4)

    nc = bacc.Bacc("TRN2", target_bir_lowering=False, debug=False,
                   num_devices=N_CORES)

    # ---- I/O ----
    xt = nc.dram_tensor("xt", [C, CH], F32, kind="ExternalInput").ap()
    wq8 = nc.dram_tensor("wq8", [128, 2 * NB * 2, 128], F8, kind="ExternalInput").ap()
    wk8 = nc.dram_tensor("wk8", [128, 2 * NB * 2, 128], F8, kind="ExternalInput").ap()
    wv8 = nc.dram_tensor("wv8", [128, NB * 2, 256], F8, kind="ExternalInput").ap()
    bq = nc.dram_tensor("bq", [256, 1], F32, kind="ExternalInput").ap()
    bk = nc.dram_tensor("bk", [256, 1], F32, kind="ExternalInput").ap()
    bv_bc = nc.dram_tensor("bv_bc", [128, 256], F32, kind="ExternalInput").ap()
    wproj8 = nc.dram_tensor("wproj8", [128, NCT * NB * 2, 128], F8,
                            kind="ExternalInput").ap()
    bproj = nc.dram_tensor("bproj", [C, 1], F32, kind="ExternalInput").ap()
    wf18 = nc.dram_tensor("wf18", [NFT, 128, NB * 2, 128], F8,
                          kind="ExternalInput").ap()
    bf1 = nc.dram_tensor("bf1", [F, 1], F32, kind="ExternalInput").ap()
    wf28 = nc.dram_tensor("wf28", [NCT, 128, NFB * 2, 128], F8,
                          kind="ExternalInput").ap()
    bf2 = nc.dram_tensor("bf2", [C, 1], F32, kind="ExternalInput").ap()
    masks = nc.dram_tensor("masks", [128, 4, QS], BF16, kind="ExternalInput").ap()
    out = nc.dram_tensor("out", [C, CH], F32, kind="ExternalOutput").ap()

    RG = [list(range(N_CORES))]

    with tile.TileContext(nc) as tc:
        with tc.tile_pool(name="dram", bufs=1, space="DRAM") as dram, \
             tc.tile_pool(name="persist", bufs=1) as persist:
            n1_bounce = [dram.tile([C // 2, CH], F8, tag=f"n1_bounce{hh}",
                                   name="n1_bounce") for hh in range(2)]
            n1_full = [dram.tile([N_CORES * C // 2, CH], F8, tag=f"n1_full{hh}",
                                 name="n1_full", addr_space="Shared")
                       for hh in range(2)]
            a2a_in = [dram.tile([C // 2, CH], F8, tag=f"a2a_in{h}",
                                name="a2a_in") for h in range(2)]
            a2a_out = [dram.tile([C // 2, CH], F8, tag=f"a2a_out{h}",
                                 name="a2a_out") for h in range(2)]

            # x tiles are the critical path at t=0: issue their DMAs first.
            xt_view = xt.rearrange("(k p) t -> p k t", p=128)
            ones_sq = persist.tile([128, 128], F32, tag="ones_sq", name="ones_sq")
            ones_bf = persist.tile([128, 128], BF16, tag="ones_bf", name="ones_bf")
            nc.vector.memset(ones_sq[:], 1.0)
            nc.vector.tensor_copy(ones_bf[:], ones_sq[:])
            masks_sb = persist.tile([128, 4, QS], BF16, tag="masks", name="masks_sb")
            nc.sync.dma_start(masks_sb[:], masks)
            bq_sb = persist.tile([128, 2, 1], F32, tag="bq", name="bq_sb")
            bk_sb = persist.tile([128, 2, 1], F32, tag="bk", name="bk_sb")
            nc.sync.dma_start(bq_sb[:], bq.rearrange("(o p) u -> p o u", p=128))
            nc.sync.dma_start(bk_sb[:], bk.rearrange("(o p) u -> p o u", p=128))
            bv_sb = persist.tile([128, 256], F32, tag="bv", name="bv_sb")
            nc.sync.dma_start(bv_sb[:], bv_bc)
            bproj_sb = persist.tile([128, NCT, 1], F32, tag="bproj", name="bproj_sb")
            nc.sync.dma_start(bproj_sb[:], bproj.rearrange("(o p) u -> p o u", p=128))
            bf1_sb = persist.tile([128, NFT, 1], F32, tag="bf1", name="bf1_sb")
            nc.sync.dma_start(bf1_sb[:], bf1.rearrange("(o p) u -> p o u", p=128))
            bf2_sb = persist.tile([128, NCT, 1], F32, tag="bf2", name="bf2_sb")
            nc.sync.dma_start(bf2_sb[:], bf2.rearrange("(o p) u -> p o u", p=128))

            # r1 survives proj -> final residual add; x survives LN1 -> proj.
            r1_sb = persist.tile([128, NCT, CH], F32, tag="r1", name="r1_sb")

            with tc.tile_pool(name="xpool", bufs=1) as xpool:
                # four separate tiles: a single x tile would serialize each
                # quarter's DMA behind the previous quarter's readers (the
                # dependency tracker is tile-granular)
                x_q = [xpool.tile([128, 4, CH], F32, tag=f"x_q{q}", name="x_q")
                       for q in range(4)]

                def x_tile(k):
                    return x_q[k // 4][:, k % 4, :]
                n2pool = tc.alloc_tile_pool(name="n2pool", bufs=1)
                n2_sb = n2pool.tile([128, NCT, CH], F8, tag="n2_sb",
                                    name="n2_sb")
                projw = tc.alloc_tile_pool(name="projw", bufs=1)
                wp_sb = projw.tile([128, NCT * NB * 2, 128], F8, tag="wp",
                                   name="wp_sb")
                wqkvp = tc.alloc_tile_pool(name="wqkv", bufs=1)
                wq_sb = wqkvp.tile([128, 2 * NB * 2, 128], F8, tag="wq",
                                   name="wq_sb")
                wk_sb = wqkvp.tile([128, 2 * NB * 2, 128], F8, tag="wk",
                                   name="wk_sb")
                wv_sb = wqkvp.tile([128, NB * 2, 256], F8, tag="wv",
                                   name="wv_sb")
                xbfpool = tc.alloc_tile_pool(name="xbfpool", bufs=1)
                x_bf = xbfpool.tile([128, NCT, CH], BF16, tag="x_bf", name="x_bf")

                # ================= Phase A: LN1 on own chunk =================
                with tc.tile_pool(name="lnA", bufs=3) as lnA, \
                     tc.tile_pool(name="lnA_small", bufs=1) as lnAs, \
                     tc.tile_pool(name="n1pool", bufs=2) as n1pool, \
                     tc.tile_pool(name="psA", bufs=1, space="PSUM") as psA:
                    ps_sum = psA.tile([128, CH], F32, tag="sum", name="ps_sum")
                    ps_ssq = psA.tile([128, CH], F32, tag="ssq", name="ps_ssq")
                    for q in range(4):
                        nc.sync.dma_start(x_q[q][:], xt_view[:, 4 * q:4 * (q + 1), :])
                    for k in range(NCT):
                        nc.scalar.activation(x_bf[:, k, :], x_tile(k),
                                             AF.Identity)
                        sq = lnA.tile([128, CH], BF16, tag="sq", name="sq")
                        nc.vector.tensor_mul(sq[:], x_bf[:, k, :], x_bf[:, k, :])
                        nc.tensor.matmul(ps_sum[:], ones_bf[:], x_bf[:, k, :],
                                         start=(k == 0), stop=(k == NCT - 1))
                        nc.tensor.matmul(ps_ssq[:], ones_bf[:], sq[:],
                                         start=(k == 0), stop=(k == NCT - 1))
                    # queue the QKV weight loads behind the x tiles, ahead of
                    # the dep-gated bounce writes (DMA queue is FIFO)
                    nc.sync.dma_start(wq_sb[:], wq8)
                    nc.sync.dma_start(wk_sb[:], wk8)
                    nc.sync.dma_start(wv_sb[:], wv8)
                    rstd_bf, shift_bf = _ln_finish(nc, lnAs, ps_sum, ps_ssq,
                                                   C, CH, "ln1", lp=nc)
                    n1_views = [n1_bounce[hh][:].rearrange("(k p) t -> p k t",
                                                           p=128)
                                for hh in range(2)]
                    # applies write into quarter staging tiles; one bounce DMA
                    # per quarter (per-tile DMAs pay ~0.7us dispatch each).
                    # Pool takes every 3rd tile (its ops cost ~2.6x DVE's).
                    for q in range(4):
                        n1s = n1pool.tile([128, 4, CH], F8, tag="n1s",
                                          name="n1s")
                        for kk in range(4):
                            k = 4 * q + kk
                            eng = nc.gpsimd if kk == 3 else nc.vector
                            tmp = lnA.tile([128, CH], BF16,
                                           tag="apP" if kk == 3 else "apV",
                                           name="tmp")
                            eng.tensor_mul(tmp[:], x_bf[:, k, :], rstd_bf[:])
                            eng.tensor_add(n1s[:, kk, :], tmp[:], shift_bf[:])
                        nc.sync.dma_start(
                            n1_views[q // 2][:, 4 * (q % 2):4 * (q % 2) + 4, :],
                            n1s[:])
                xbfpool.release()

                for hh in range(2):
                    if collectives:
                        nc.gpsimd.collective_compute(
                            "AllGather", OP.bypass, replica_groups=RG,
                            ins=[n1_bounce[hh].opt()], outs=[n1_full[hh].opt()])
                    else:  # timing variant: plain copy keeps the dependency edge
                        nc.sync.dma_start(n1_full[hh][0:C // 2, :],
                                          n1_bounce[hh][:])

                # ====== Phase B: QKV (all tokens, own 2 heads, fp8 DR) ======
                with tc.tile_pool(name="qkv_sb", bufs=1) as qkvp:
                    q_sb = qkvp.tile([128, 2, NTOK], BF16, tag="q_sb", name="q_sb")
                    k_sb = qkvp.tile([128, 2, NTOK], BF16, tag="k_sb", name="k_sb")
                    v_sb = qkvp.tile([128, NVT, 256], BF16, tag="v_sb", name="v_sb")

                    with tc.tile_pool(name="n1t", bufs=3) as n1tp, \
                         tc.tile_pool(name="psQK", bufs=1, space="PSUM") as psQK, \
                         tc.tile_pool(name="psV", bufs=1, space="PSUM") as psV:
                        nf_views = [n1_full[hh][:].rearrange(
                            "(r k p) t -> r p k t", r=N_CORES, p=128)
                            for hh in range(2)]
                        for r in range(N_CORES):
                            n1ca = n1tp.tile([128, NCT // 2, CH], F8, tag="n1ca",
                                             name="n1ca")
                            nc.sync.dma_start(n1ca[:], nf_views[0][r])
                            n1cb = n1tp.tile([128, NCT // 2, CH], F8, tag="n1cb",
                                             name="n1cb")
                            nc.sync.dma_start(n1cb[:], nf_views[1][r])
                            ps_q = [psQK.tile([128, CH], F32, tag=f"q{o}",
                                              name=f"ps_q{o}") for o in range(2)]
                            ps_k = [psQK.tile([128, CH], F32, tag=f"k{o}",
                                              name=f"ps_k{o}") for o in range(2)]
                            ps_v = [psV.tile([128, 256], F32, tag=f"v{s}",
                                             name=f"ps_v{s}") for s in range(S_SUB)]
                            for b in range(NB):
                                n1c = n1ca if b < NB // 2 else n1cb
                                bl = b % (NB // 2)
                                rhs = n1c[:, 2 * bl:2 * bl + 2, :]
                                st, sp = (b == 0), (b == NB - 1)
                                for o in range(2):
                                    nc.tensor.matmul(
                                        ps_q[o][:],
                                        wq_sb[:, (o * NB + b) * 2:(o * NB + b) * 2 + 2, :],
                                        rhs, start=st, stop=sp, perf_mode=PM.DoubleRow)
                                    nc.tensor.matmul(
                                        ps_k[o][:],
                                        wk_sb[:, (o * NB + b) * 2:(o * NB + b) * 2 + 2, :],
                                        rhs, start=st, stop=sp, perf_mode=PM.DoubleRow)
                                for s in range(S_SUB):
                                    nc.tensor.matmul(
                                        ps_v[s][:],
                                        n1c[:, 2 * bl:2 * bl + 2, 128 * s:128 * (s + 1)],
                                        wv_sb[:, 2 * b:2 * b + 2, :],
                                        start=st, stop=sp, perf_mode=PM.DoubleRow)
                            for o in range(2):
                                nc.vector.tensor_scalar(
                                    q_sb[:, o, CH * r:CH * (r + 1)], ps_q[o][:],
                                    RWS, bq_sb[:, o, :], op0=OP.mult, op1=OP.add)
                                nc.vector.tensor_scalar(
                                    k_sb[:, o, CH * r:CH * (r + 1)], ps_k[o][:],
                                    RWS, bk_sb[:, o, :], op0=OP.mult, op1=OP.add)
                            for s in range(S_SUB):
                                nc.vector.scalar_tensor_tensor(
                                    v_sb[:, r * S_SUB + s, :], ps_v[s][:], RWS,
                                    bv_sb[:], op0=OP.mult, op1=OP.add)

                    # ========= Phase B2: attention per (head, batch) =========
                    WPC = NCT * NB * 2 // 8
                    for pc in range(8):
                        nc.sync.dma_start(wp_sb[:, WPC * pc:WPC * (pc + 1), :],
                                          wproj8[:, WPC * pc:WPC * (pc + 1), :])
                    with tc.tile_pool(name="attn_e", bufs=4) as ep, \
                         tc.tile_pool(name="attn_acc", bufs=2) as accp, \
                         tc.tile_pool(name="attn_small", bufs=3) as asml, \
                         tc.tile_pool(name="vals", bufs=3) as valsp, \
                         tc.tile_pool(name="psS", bufs=2, space="PSUM") as psS, \
                         tc.tile_pool(name="psAV", bufs=2, space="PSUM") as psAV, \
                         tc.tile_pool(name="psDen", bufs=2, space="PSUM") as psDen:
                        for h in range(H_PER_CORE):
                            for bb in range(B):
                                for j in range(NQS):
                                    ni = 4 * (j + 1) if QS == 512 else T // 128
                                    ps_av = psAV.tile([128, QS], F32, tag="av",
                                                      name="ps_av")
                                    ps_den = psDen.tile([128, QS], F32, tag="den",
                                                        name="ps_den")
                                    e_acc = accp.tile([128, QS], BF16, tag="eacc",
                                                      name="e_acc")
                                    qtok = bb * T + j * QS
                                    for u in range(ni // 2):
                                        # paired score tiles share one Exp call
                                        # over [128, 1024] (amortizes Act setup)
                                        ps_s2 = psS.tile([128, 2, QS], F32,
                                                         tag="s2", name="ps_s2")
                                        for hf in range(2):
                                            i = 2 * u + hf
                                            ktok = bb * T + i * 128
                                            nc.tensor.matmul(
                                                ps_s2[:, hf, :],
                                                k_sb[:, h, ktok:ktok + 128],
                                                q_sb[:, h, qtok:qtok + QS],
                                                start=True, stop=True)
                                        e2 = ep.tile([128, 2, QS], BF16, tag="e2",
                                                     name="e2")
                                        nc.scalar.activation(e2[:], ps_s2[:],
                                                             AF.Exp, bias=0.0,
                                                             scale=SCALE)
                                        d0 = 2 * u - (ni - 4)
                                        if d0 >= 0:
                                            nc.vector.tensor_mul(
                                                e2[:], e2[:],
                                                masks_sb[:, d0:d0 + 2, :])
                                        for hf in range(2):
                                            i = 2 * u + hf
                                            # hybrid denominator: diagonal tiles
                                            # accumulate on PE, the rest on DVE
                                            if i < ni - 4:
                                                if i == 0:
                                                    nc.vector.tensor_copy(
                                                        e_acc[:], e2[:, hf, :])
                                                else:
                                                    nc.vector.tensor_add(
                                                        e_acc[:], e_acc[:],
                                                        e2[:, hf, :])
                                            else:
                                                nc.tensor.matmul(
                                                    ps_den[:], ones_bf[:],
                                                    e2[:, hf, :],
                                                    start=(i == ni - 4),
                                                    stop=(i == ni - 1 and ni == 4))
                                            tt = (bb * T + i * 128) // 128
                                            nc.tensor.matmul(
                                                ps_av[:],
                                                v_sb[:, tt, 128 * h:128 * (h + 1)],
                                                e2[:, hf, :],
                                                start=(i == 0), stop=(i == ni - 1))
                                    if ni > 4:
                                        nc.tensor.matmul(ps_den[:], ones_bf[:],
                                                         e_acc[:], start=False,
                                                         stop=True)
                                    rec_bc = asml.tile([128, QS], F32, tag="rec",
                                                       name="rec_bc")
                                    nc.vector.reciprocal(rec_bc[:], ps_den[:])
                                    vtile = valsp.tile([128, QS], F8, tag="vt",
                                                       name="vtile")
                                    nc.vector.tensor_mul(vtile[:], ps_av[:],
                                                         rec_bc[:])
                                    ncol0 = bb * T + j * QS
                                    for part in range(max(1, QS // CH)):
                                        jg = (ncol0 + part * CH) // CH
                                        w = min(CH, QS)
                                        nc.sync.dma_start(
                                            a2a_in[h][128 * jg:128 * (jg + 1), :],
                                            vtile[:, part * w:(part + 1) * w])
                            if h == 0:
                                # h=0 values complete at half-time: overlap the
                                # first AllToAll with the h=1 attention pass
                                if collectives:
                                    nc.gpsimd.collective_compute(
                                        "AllToAll", OP.bypass, replica_groups=RG,
                                        ins=[a2a_in[0].opt()],
                                        outs=[a2a_out[0].opt()])
                                else:
                                    nc.sync.dma_start(a2a_out[0][:], a2a_in[0][:])

                wqkvp.release()
                if collectives:
                    nc.gpsimd.collective_compute(
                        "AllToAll", OP.bypass, replica_groups=RG,
                        ins=[a2a_in[1].opt()], outs=[a2a_out[1].opt()])
                else:
                    nc.sync.dma_start(a2a_out[1][:], a2a_in[1][:])

                # ====== Phase C: proj + residual + LN2 stats (own chunk) ======
                with tc.tile_pool(name="vf", bufs=1) as vfp, \
                     tc.tile_pool(name="pdrain", bufs=3) as pdp, \
                     tc.tile_pool(name="lnC_small", bufs=1) as lnCs, \
                     tc.tile_pool(name="psP", bufs=3, space="PSUM") as psP, \
                     tc.tile_pool(name="psP2", bufs=1, space="PSUM") as psP2:
                    vf_sb = vfp.tile([128, NB, 2, CH], F8, tag="vf",
                                     name="vf_sb")
                    for hs in range(2):
                        nc.sync.dma_start(
                            vf_sb[:, :, hs, :],
                            a2a_out[hs][:].rearrange("(r p) t -> p r t", p=128))
                    ps_sum2 = psP2.tile([128, CH], F32, tag="sum2", name="ps_sum2")
                    ps_ssq2 = psP2.tile([128, CH], F32, tag="ssq2", name="ps_ssq2")
                    r1bfp = tc.alloc_tile_pool(name="r1bf", bufs=1)
                    r1_bf = r1bfp.tile([128, NCT, CH], BF16, tag="r1_bf",
                                       name="r1_bf")

                    def ln2_stats(ot):
                        # lag-2 interleave behind the proj loop: r1[ot] is ready
                        # two iterations later, so the PE stats matmuls never
                        # stall on the drain chain
                        if ot % 2 == 0:
                            nc.scalar.activation(r1_bf[:, ot, :], r1_sb[:, ot, :],
                                                 AF.Identity)
                        else:
                            nc.vector.tensor_copy(r1_bf[:, ot, :],
                                                  r1_sb[:, ot, :])
                        sq2 = pdp.tile([128, CH], BF16, tag="sq2", name="sq2")
                        nc.vector.tensor_mul(sq2[:], r1_bf[:, ot, :],
                                             r1_bf[:, ot, :])
                        nc.tensor.matmul(ps_sum2[:], ones_bf[:], r1_bf[:, ot, :],
                                         start=(ot == 0), stop=(ot == NCT - 1))
                        nc.tensor.matmul(ps_ssq2[:], ones_bf[:], sq2[:],
                                         start=(ot == 0), stop=(ot == NCT - 1))

                    for ot in range(NCT):
                        ps_p = psP.tile([128, CH], F32, tag="p", name="ps_p")
                        for b in range(NB):
                            nc.tensor.matmul(
                                ps_p[:],
                                wp_sb[:, (ot * NB + b) * 2:(ot * NB + b) * 2 + 2, :],
                                vf_sb[:, b, :, :],
                                start=(b == 0), stop=(b == NB - 1),
                                perf_mode=PM.DoubleRow)
                        p_t = pdp.tile([128, CH], F32, tag="p_t", name="p_t")
                        nc.scalar.activation(p_t[:], ps_p[:], AF.Identity,
                                             bias=bproj_sb[:, ot, :], scale=RWS)
                        nc.vector.tensor_add(r1_sb[:, ot, :], p_t[:],
                                             x_tile(ot))
                        if ot >= 2:
                            ln2_stats(ot - 2)
                    ln2_stats(NCT - 2)
                    ln2_stats(NCT - 1)
                    rstd2_bf, shift2_bf = _ln_finish(nc, lnCs, ps_sum2, ps_ssq2,
                                                     C, CH, "ln2", lp=nc)
                    for k in range(NCT):
                        if k % 3 == 2:
                            tmp2 = pdp.tile([128, CH], BF16, tag="ap2P",
                                            name="tmp2")
                            nc.gpsimd.tensor_mul(tmp2[:], r1_bf[:, k, :],
                                                 rstd2_bf[:])
                            nc.gpsimd.tensor_add(n2_sb[:, k, :], tmp2[:],
                                                 shift2_bf[:])
                        else:
                            tmp2 = pdp.tile([128, CH], BF16, tag="ap2V",
                                            name="tmp2")
                            nc.vector.tensor_mul(tmp2[:], r1_bf[:, k, :],
                                                 rstd2_bf[:])
                            nc.vector.tensor_add(n2_sb[:, k, :], tmp2[:],
                                                 shift2_bf[:])
                    r1bfp.release()

                projw.release()
                # =============== Phase D: FFN1 (fp8 DR) ===============
                with tc.tile_pool(name="hpool", bufs=1) as hpool, \
                     tc.tile_pool(name="w1", bufs=6) as w1p, \
                     tc.tile_pool(name="psH", bufs=3, space="PSUM") as psH:
                    h_sb = hpool.tile([128, NFT, CH], F8, tag="h_sb",
                                      name="h_sb")
                    for ft in range(NFT):
                        w1t = w1p.tile([128, NB * 2, 128], F8, tag="w1",
                                       name="w1t")
                        nc.sync.dma_start(w1t[:], wf18[ft])
                        ps_h = psH.tile([128, CH], F32, tag="h", name="ps_h")
                        for b in range(NB):
                            nc.tensor.matmul(
                                ps_h[:], w1t[:, 2 * b:2 * b + 2, :],
                                n2_sb[:, 2 * b:2 * b + 2, :],
                                start=(b == 0), stop=(b == NB - 1),
                                perf_mode=PM.DoubleRow)
                        nc.scalar.activation(h_sb[:, ft, :], ps_h[:], GELU,
                                             bias=bf1_sb[:, ft, :], scale=RWS)

                    # ============= Phase D2: FFN2 (fp8 DR) =============
                    with tc.tile_pool(name="w2", bufs=3) as w2p, \
                         tc.tile_pool(name="outp", bufs=3) as outp, \
                         tc.tile_pool(name="psF", bufs=2, space="PSUM") as psF:
                        for ot in range(NCT):
                            w2t = w2p.tile([128, NFB * 2, 128], F8, tag="w2",
                                           name="w2t")
                            nc.sync.dma_start(w2t[:], wf28[ot])
                            ps_f = psF.tile([128, CH], F32, tag="f",
                                            name="ps_f")
                            for g in range(NFB):
                                nc.tensor.matmul(
                                    ps_f[:], w2t[:, 2 * g:2 * g + 2, :],
                                    h_sb[:, 2 * g:2 * g + 2, :],
                                    start=(g == 0), stop=(g == NFB - 1),
                                    perf_mode=PM.DoubleRow)
                            f_t = outp.tile([128, CH], F32, tag="f_t",
                                            name="f_t")
                            nc.scalar.activation(f_t[:], ps_f[:], AF.Identity,
                                                 bias=bf2_sb[:, ot, :],
                                                 scale=RWS)
                            o_t = outp.tile([128, CH], F32, tag="o_t",
                                            name="o_t")
                            nc.vector.tensor_add(o_t[:], f_t[:],
                                                 r1_sb[:, ot, :])
                            nc.sync.dma_start(out[128 * ot:128 * (ot + 1), :],
                                              o_t[:])
                n2pool.release()

    nc.compile()
    return nc


# ----------------------------------------------------------------------------
# Host side
# ----------------------------------------------------------------------------

_NC_CACHE = {}


def _get_nc(T=2048):
    if T not in _NC_CACHE:
        _NC_CACHE[T] = build_decoder(T)
    return _NC_CACHE[T]


def _q8(a):
    """Quantize f32 -> fp8 e4m3 bytes with the x32 pre-scale."""
    return (np.ascontiguousarray(a, np.float32) * WS).astype(
        ml_dtypes.float8_e4m3).view(np.uint8)


def _dr_lhsT_flat(W):
    """[K, M] f32 -> DoubleRow lhsT SBUF layout [128, (M/128 * K/256 * 2), 128]."""
    K, M = W.shape
    a = W.reshape(K // 256, 2, 128, M // 128, 128).transpose(2, 3, 0, 1, 4)
    return np.ascontiguousarray(a.reshape(128, (M // 128) * (K // 256) * 2, 128))


def _dr_lhsT_tiles(W):
    """[K, M] f32 -> per-out-tile DoubleRow layout [M/128, 128, K/256 * 2, 128]."""
    K, M = W.shape
    a = W.reshape(K // 256, 2, 128, M // 128, 128).transpose(3, 2, 0, 1, 4)
    return np.ascontiguousarray(a.reshape(M // 128, 128, (K // 256) * 2, 128))


def _bf16(a):
    return np.ascontiguousarray(a).astype(ml_dtypes.bfloat16).view(np.uint16)


def _prep_inputs(x, Wqkv, bqkv, Wproj, bproj, Wf1, bf1, Wf2, bf2,
                 g1, b1, g2, b2):
    """Fold LN affines, slice heads per core, build per-core in_maps."""
    f32 = np.float32
    x = np.asarray(x, f32)
    Bx, T, Cx = x.shape
    NTOK = Bx * T
    CH = NTOK // N_CORES
    Wqkv = np.asarray(Wqkv, f32)
    bqkv = np.asarray(bqkv, f32)
    g1 = np.asarray(g1, f32); b1 = np.asarray(b1, f32)
    g2 = np.asarray(g2, f32); b2 = np.asarray(b2, f32)
    Wqkv_eff = g1[:, None] * Wqkv
    bqkv_eff = b1 @ Wqkv + bqkv
    Wf1 = np.asarray(Wf1, f32)
    bf1v = np.asarray(bf1, f32)
    Wf1_eff = g2[:, None] * Wf1
    bf1_eff = b2 @ Wf1 + bf1v
    Wproj = np.asarray(Wproj, f32)
    bprojv = np.asarray(bproj, f32)
    Wf2 = np.asarray(Wf2, f32)
    bf2v = np.asarray(bf2, f32)

    xt = np.ascontiguousarray(x.reshape(NTOK, Cx).T)        # [C, NT]

    QS = min(512, T)
    masks = np.zeros((128, 4, QS), f32)
    p = np.arange(128)[:, None]
    fcol = np.arange(QS)[None, :]
    for m in range(4):
        masks[:, m, :] = (p <= fcol - 128 * m).astype(f32)

    shared = {
        "wproj8": _q8(_dr_lhsT_flat(Wproj)),
        "bproj": bprojv.reshape(Cx, 1),
        "wf18": _q8(_dr_lhsT_tiles(Wf1_eff)),
        "bf1": bf1_eff.reshape(F, 1),
        "wf28": _q8(_dr_lhsT_tiles(Wf2)),
        "bf2": bf2v.reshape(Cx, 1),
        "masks": _bf16(masks),
    }
    in_maps = []
    for c in range(N_CORES):
        h0, h1 = 2 * c, 2 * c + 1
        qcols = np.concatenate([h0 * 384 + np.arange(128),
                                h1 * 384 + np.arange(128)])
        kcols = qcols + 128
        vcols = qcols + 256
        m = dict(shared)
        m["xt"] = np.ascontiguousarray(xt[:, c * CH:(c + 1) * CH])
        m["wq8"] = _q8(_dr_lhsT_flat(Wqkv_eff[:, qcols]))
        m["wk8"] = _q8(_dr_lhsT_flat(Wqkv_eff[:, kcols]))
        # v rhs SBUF layout: [128, K/256 * 2, 256]
        wv = Wqkv_eff[:, vcols].reshape(8, 2, 128, 256).transpose(2, 0, 1, 3)
        m["wv8"] = _q8(np.ascontiguousarray(wv.reshape(128, 16, 256)))
        m["bq"] = np.ascontiguousarray(bqkv_eff[qcols].reshape(256, 1))
        m["bk"] = np.ascontiguousarray(bqkv_eff[kcols].reshape(256, 1))
        m["bv_bc"] = np.ascontiguousarray(
            np.broadcast_to(bqkv_eff[vcols][None, :], (128, 256)))
        in_maps.append(m)
    return in_maps, (Bx, T, Cx, CH)


def kernel(x, Wqkv, bqkv, Wproj, bproj, Wf1, bf1, Wf2, bf2,
           g1, b1, g2, b2, _trace=False):
    in_maps, (Bx, T, Cx, CH) = _prep_inputs(
        x, Wqkv, bqkv, Wproj, bproj, Wf1, bf1, Wf2, bf2, g1, b1, g2, b2)
    nc = _get_nc(T)
    res = bass_utils.run_bass_kernel_spmd(
        nc, in_maps, core_ids=list(range(N_CORES)), trace=_trace)
    kernel.last_results = res
    NTOK = Bx * T
    out_t = np.empty((NTOK, Cx), np.float32)
    for c in range(N_CORES):
        out_t[c * CH:(c + 1) * CH, :] = res.results[c]["out"].T
    return out_t.reshape(Bx, T, Cx)


# revision 17
# speedup vs baseline: 1.1066x; 1.0605x over previous
"""Trainium2 Bass kernel for a dense decoder block (LN->MHA->res, LN->FFN->res).

Sharding (8 cores, one NEFF, SPMD-uniform addressing):
  - LN1 token-parallel (512-token chunk/core) -> AllGather of normalized acts
    quantized to fp8e4m3 (x32-scaled weights keep fp8 out of subnormals).
  - QKV + attention head-parallel (2 heads/core, causal, unstable softmax --
    exact because masked logits multiply to 0 post-exp).
  - AllToAll redistributes attention values (fp8): head-shards -> token-shards.
  - proj + residual + LN2 + FFN token-parallel with fp8 weights streamed.
  - LN affine params are folded into the following matmul weights on host.

All heavy GEMMs run in fp8e4m3 with MatmulPerfMode.DoubleRow (256-row
contraction per instruction, 0.5 cycles/output column -> 4x f32r MACs/cycle).
Weights are pre-scaled by 32 on host; PSUM drains apply 1/32 via the
activation-engine scale. Attention (scores/softmax/AV) runs in bf16.
LN statistics use the ones-matmul trick in f32r (1 cycle/row).
Activations stay channel-major [C, tokens]; v is produced token-major
directly by swapping matmul operands, so no transposes are needed.
"""

import math

import numpy as np
import ml_dtypes

import concourse.bass as bass
import concourse.mybir as mybir
import concourse.tile as tile
from concourse import bacc
from concourse import bass_utils

F32 = mybir.dt.float32
F32R = mybir.dt.float32r
BF16 = mybir.dt.bfloat16
F8 = mybir.dt.float8e4
AF = mybir.ActivationFunctionType
OP = mybir.AluOpType
PM = mybir.MatmulPerfMode

N_CORES = 8
B = 2
C = 2048
H = 16
HD = 128
F = 8192
NT = B * 2048                       # total tokens (B*T with T=2048)
H_PER_CORE = H // N_CORES           # 2
NCT = C // 128                      # 16 channel tiles
NB = C // 256                       # 8 DoubleRow contraction blocks
NFT = F // 128                      # 64 ffn tiles
NFB = F // 256                      # 32 ffn DoubleRow blocks
EPS = 1e-5
SCALE = 1.0 / math.sqrt(HD)
WS = 32.0                           # host-side weight scale (drains apply 1/WS)
RWS = 1.0 / WS
GELU = AF.Gelu_apprx_tanh


def r32(ap):
    return ap.bitcast(F32R)


def _ln_finish(nc, pool_small, ps_sum, ps_ssq, n_tok, ncols, tagpfx, lp=None):
    """From broadcast sum/sumsq psums produce SBUF rstd/shift [128, ncols].

    With lp set, outputs are bf16 (feeding the bf16 apply path)."""
    odt = BF16 if lp is not None else F32
    mean = pool_small.tile([128, ncols], F32, tag=f"{tagpfx}_mean", name="mean")
    ex2 = pool_small.tile([128, ncols], F32, tag=f"{tagpfx}_ex2", name="ex2")
    nc.vector.tensor_scalar_mul(mean[:], ps_sum[:], 1.0 / n_tok)
    nc.vector.tensor_scalar_mul(ex2[:], ps_ssq[:], 1.0 / n_tok)
    msq = pool_small.tile([128, ncols], F32, tag=f"{tagpfx}_msq", name="msq")
    nc.vector.tensor_mul(msq[:], mean[:], mean[:])
    varp = pool_small.tile([128, ncols], F32, tag=f"{tagpfx}_varp", name="varp")
    nc.vector.scalar_tensor_tensor(varp[:], ex2[:], EPS, msq[:],
                                   op0=OP.add, op1=OP.subtract)
    std = pool_small.tile([128, ncols], F32, tag=f"{tagpfx}_std", name="std")
    nc.scalar.sqrt(std[:], varp[:])
    rstd_bc = pool_small.tile([128, ncols], odt, tag=f"{tagpfx}_rstd", name="rstd")
    if lp is not None:
        with lp.allow_low_precision(reason="rstd broadcast feeds fp8 path"):
            nc.vector.reciprocal(rstd_bc[:], std[:])
    else:
        nc.vector.reciprocal(rstd_bc[:], std[:])
    shift_bc = pool_small.tile([128, ncols], odt, tag=f"{tagpfx}_shift", name="shift")
    nc.vector.scalar_tensor_tensor(shift_bc[:], mean[:], -1.0, rstd_bc[:],
                                   op0=OP.mult, op1=OP.mult)
    return rstd_bc, shift_bc


def build_decoder(T=2048, collectives=True):
    """Build the SPMD decoder-block program for seq length T (2048 = real)."""
    NTOK = B * T
    CH = NTOK // N_CORES            # tokens per core chunk (512)
    NQS = max(1, T // 512)          # q slices of 512 per batch elem
    QS = min(512, T)
    NVT = NTOK // 128               # token-major v tiles (32)
    S_SUB = CH // 128               # 128-token subtiles per chunk (4)

    nc = bacc.Bacc("TRN2", target_bir_lowering=False, debug=False,
                   num_devices=N_CORES)

    # ---- I/O ----
    xt = nc.dram_tensor("xt", [C, CH], F32, kind="ExternalInput").ap()
    wq8 = nc.dram_tensor("wq8", [128, 2 * NB * 2, 128], F8, kind="ExternalInput").ap()
    wk8 = nc.dram_tensor("wk8", [128, 2 * NB * 2, 128], F8, kind="ExternalInput").ap()
    wv8 = nc.dram_tensor("wv8", [128, NB * 2, 256], F8, kind="ExternalInput").ap()
    bq = nc.dram_tensor("bq", [128, 2, 1], F32, kind="ExternalInput").ap()
    bk = nc.dram_tensor("bk", [128, 2, 1], F32, kind="ExternalInput").ap()
    bv_bc = nc.dram_tensor("bv_bc", [128, 256], F32, kind="ExternalInput").ap()
    wproj8 = nc.dram_tensor("wproj8", [128, NCT * NB * 2, 128], F8,
                            kind="ExternalInput").ap()
    bproj = nc.dram_tensor("bproj", [128, NCT, 1], F32, kind="ExternalInput").ap()
    wf18 = nc.dram_tensor("wf18", [NFT, 128, NB * 2, 128], F8,
                          kind="ExternalInput").ap()
    bf1 = nc.dram_tensor("bf1", [128, NFT, 1], F32, kind="ExternalInput").ap()
    wf28 = nc.dram_tensor("wf28", [NCT, 128, NFB * 2, 128], F8,
                          kind="ExternalInput").ap()
    bf2 = nc.dram_tensor("bf2", [128, NCT, 1], F32, kind="ExternalInput").ap()
    masks = nc.dram_tensor("masks", [128, 4, QS], BF16, kind="ExternalInput").ap()
    out = nc.dram_tensor("out", [C, CH], F32, kind="ExternalOutput").ap()

    RG = [list(range(N_CORES))]

    with tile.TileContext(nc) as tc:
        with tc.tile_pool(name="dram", bufs=1, space="DRAM") as dram, \
             tc.tile_pool(name="persist", bufs=1) as persist:
            n1_bounce = [dram.tile([C // 2, CH], F8, tag=f"n1_bounce{hh}",
                                   name="n1_bounce") for hh in range(2)]
            n1_full = [dram.tile([N_CORES * C // 2, CH], F8, tag=f"n1_full{hh}",
                                 name="n1_full", addr_space="Shared")
                       for hh in range(2)]
            a2a_in = [dram.tile([C // 2, CH], F8, tag=f"a2a_in{h}",
                                name="a2a_in") for h in range(2)]
            a2a_out = [dram.tile([C // 2, CH], F8, tag=f"a2a_out{h}",
                                 name="a2a_out") for h in range(2)]

            # x tiles are the critical path at t=0: issue their DMAs first.
            xt_view = xt.rearrange("(k p) t -> p k t", p=128)
            ones_sq = persist.tile([128, 128], F32, tag="ones_sq", name="ones_sq")
            ones_bf = persist.tile([128, 128], BF16, tag="ones_bf", name="ones_bf")
            nc.vector.memset(ones_sq[:], 1.0)
            nc.vector.tensor_copy(ones_bf[:], ones_sq[:])
            masks_sb = persist.tile([128, 4, QS], BF16, tag="masks", name="masks_sb")
            bq_sb = persist.tile([128, 2, 1], F32, tag="bq", name="bq_sb")
            bk_sb = persist.tile([128, 2, 1], F32, tag="bk", name="bk_sb")
            bv_sb = persist.tile([128, 256], F32, tag="bv", name="bv_sb")
            bproj_sb = persist.tile([128, NCT, 1], F32, tag="bproj", name="bproj_sb")
            bf1_sb = persist.tile([128, NFT, 1], F32, tag="bf1", name="bf1_sb")
            bf2_sb = persist.tile([128, NCT, 1], F32, tag="bf2", name="bf2_sb")

            # r1 survives proj -> final residual add; x survives LN1 -> proj.
            r1_sb = persist.tile([128, NCT, CH], F32, tag="r1", name="r1_sb")

            with tc.tile_pool(name="xpool", bufs=1) as xpool:
                # four separate tiles: a single x tile would serialize each
                # quarter's DMA behind the previous quarter's readers (the
                # dependency tracker is tile-granular)
                x_q = [xpool.tile([128, 4, CH], F32, tag=f"x_q{q}", name="x_q")
                       for q in range(4)]

                def x_tile(k):
                    return x_q[k // 4][:, k % 4, :]
                n2pool = tc.alloc_tile_pool(name="n2pool", bufs=1)
                n2_sb = n2pool.tile([128, NCT, CH], F8, tag="n2_sb",
                                    name="n2_sb")
                projw = tc.alloc_tile_pool(name="projw", bufs=1)
                wp_sb = projw.tile([128, NCT * NB * 2, 128], F8, tag="wp",
                                   name="wp_sb")
                wqkvp = tc.alloc_tile_pool(name="wqkv", bufs=1)
                wq_sb = wqkvp.tile([128, 2 * NB * 2, 128], F8, tag="wq",
                                   name="wq_sb")
                wk_sb = wqkvp.tile([128, 2 * NB * 2, 128], F8, tag="wk",
                                   name="wk_sb")
                wv_sb = wqkvp.tile([128, NB * 2, 256], F8, tag="wv",
                                   name="wv_sb")
                xbfpool = tc.alloc_tile_pool(name="xbfpool", bufs=1)
                x_bf = xbfpool.tile([128, NCT, CH], BF16, tag="x_bf", name="x_bf")

                # ================= Phase A: LN1 on own chunk =================
                with tc.tile_pool(name="lnA", bufs=3) as lnA, \
                     tc.tile_pool(name="lnA_small", bufs=1) as lnAs, \
                     tc.tile_pool(name="n1pool", bufs=2) as n1pool, \
                     tc.tile_pool(name="psA", bufs=1, space="PSUM") as psA:
                    ps_sum = psA.tile([128, CH], F32, tag="sum", name="ps_sum")
                    ps_ssq = psA.tile([128, CH], F32, tag="ssq", name="ps_ssq")
                    for q in range(4):
                        nc.sync.dma_start(x_q[q][:], xt_view[:, 4 * q:4 * (q + 1), :])
                    nc.sync.dma_start(wq_sb[:], wq8)
                    nc.sync.dma_start(wk_sb[:], wk8)
                    nc.sync.dma_start(wv_sb[:], wv8)
                    nc.sync.dma_start(bq_sb[:], bq)
                    nc.sync.dma_start(bk_sb[:], bk)
                    nc.sync.dma_start(bv_sb[:], bv_bc)
                    nc.sync.dma_start(masks_sb[:], masks)
                    nc.sync.dma_start(bproj_sb[:], bproj)
                    nc.sync.dma_start(bf1_sb[:], bf1)
                    nc.sync.dma_start(bf2_sb[:], bf2)
                    for k in range(NCT):
                        nc.scalar.activation(x_bf[:, k, :], x_tile(k),
                                             AF.Identity)
                        sq = lnA.tile([128, CH], BF16, tag="sq", name="sq")
                        nc.vector.tensor_mul(sq[:], x_bf[:, k, :], x_bf[:, k, :])
                        nc.tensor.matmul(ps_sum[:], ones_bf[:], x_bf[:, k, :],
                                         start=(k == 0), stop=(k == NCT - 1))
                        nc.tensor.matmul(ps_ssq[:], ones_bf[:], sq[:],
                                         start=(k == 0), stop=(k == NCT - 1))
                    rstd_bf, shift_bf = _ln_finish(nc, lnAs, ps_sum, ps_ssq,
                                                   C, CH, "ln1", lp=nc)
                    n1_views = [n1_bounce[hh][:].rearrange("(k p) t -> p k t",
                                                           p=128)
                                for hh in range(2)]
                    # applies write into quarter staging tiles; one bounce DMA
                    # per quarter (per-tile DMAs pay ~0.7us dispatch each).
                    # Pool takes every 3rd tile (its ops cost ~2.6x DVE's).
                    for q in range(4):
                        n1s = n1pool.tile([128, 4, CH], F8, tag="n1s",
                                          name="n1s")
                        for kk in range(4):
                            k = 4 * q + kk
                            eng = nc.gpsimd if kk == 3 else nc.vector
                            tmp = lnA.tile([128, CH], BF16,
                                           tag="apP" if kk == 3 else "apV",
                                           name="tmp")
                            eng.tensor_mul(tmp[:], x_bf[:, k, :], rstd_bf[:])
                            eng.tensor_add(n1s[:, kk, :], tmp[:], shift_bf[:])
                        nc.sync.dma_start(
                            n1_views[q // 2][:, 4 * (q % 2):4 * (q % 2) + 4, :],
                            n1s[:])
                xbfpool.release()

                for hh in range(2):
                    if collectives:
                        nc.gpsimd.collective_compute(
                            "AllGather", OP.bypass, replica_groups=RG,
                            ins=[n1_bounce[hh].opt()], outs=[n1_full[hh].opt()])
                    else:  # timing variant: plain copy keeps the dependency edge
                        nc.sync.dma_start(n1_full[hh][0:C // 2, :],
                                          n1_bounce[hh][:])

                # ====== Phase B: QKV (all tokens, own 2 heads, fp8 DR) ======
                with tc.tile_pool(name="qkv_sb", bufs=1) as qkvp:
                    q_sb = qkvp.tile([128, 2, NTOK], BF16, tag="q_sb", name="q_sb")
                    k_sb = qkvp.tile([128, 2, NTOK], BF16, tag="k_sb", name="k_sb")
                    v_sb = qkvp.tile([128, NVT, 256], BF16, tag="v_sb", name="v_sb")

                    with tc.tile_pool(name="n1t", bufs=3) as n1tp, \
                         tc.tile_pool(name="psQK", bufs=1, space="PSUM") as psQK, \
                         tc.tile_pool(name="psV", bufs=1, space="PSUM") as psV:
                        nf_views = [n1_full[hh][:].rearrange(
                            "(r k p) t -> r p k t", r=N_CORES, p=128)
                            for hh in range(2)]
                        for r in range(N_CORES):
                            n1ca = n1tp.tile([128, NCT // 2, CH], F8, tag="n1ca",
                                             name="n1ca")
                            nc.sync.dma_start(n1ca[:], nf_views[0][r])
                            n1cb = n1tp.tile([128, NCT // 2, CH], F8, tag="n1cb",
                                             name="n1cb")
                            nc.sync.dma_start(n1cb[:], nf_views[1][r])
                            ps_q = [psQK.tile([128, CH], F32, tag=f"q{o}",
                                              name=f"ps_q{o}") for o in range(2)]
                            ps_k = [psQK.tile([128, CH], F32, tag=f"k{o}",
                                              name=f"ps_k{o}") for o in range(2)]
                            ps_v = [psV.tile([128, 256], F32, tag=f"v{s}",
                                             name=f"ps_v{s}") for s in range(S_SUB)]
                            for b in range(NB):
                                n1c = n1ca if b < NB // 2 else n1cb
                                bl = b % (NB // 2)
                                rhs = n1c[:, 2 * bl:2 * bl + 2, :]
                                st, sp = (b == 0), (b == NB - 1)
                                for o in range(2):
                                    nc.tensor.matmul(
                                        ps_q[o][:],
                                        wq_sb[:, (o * NB + b) * 2:(o * NB + b) * 2 + 2, :],
                                        rhs, start=st, stop=sp, perf_mode=PM.DoubleRow)
                                    nc.tensor.matmul(
                                        ps_k[o][:],
                                        wk_sb[:, (o * NB + b) * 2:(o * NB + b) * 2 + 2, :],
                                        rhs, start=st, stop=sp, perf_mode=PM.DoubleRow)
                                for s in range(S_SUB):
                                    nc.tensor.matmul(
                                        ps_v[s][:],
                                        n1c[:, 2 * bl:2 * bl + 2, 128 * s:128 * (s + 1)],
                                        wv_sb[:, 2 * b:2 * b + 2, :],
                                        start=st, stop=sp, perf_mode=PM.DoubleRow)
                            for o in range(2):
                                nc.vector.tensor_scalar(
                                    q_sb[:, o, CH * r:CH * (r + 1)], ps_q[o][:],
                                    RWS, bq_sb[:, o, :], op0=OP.mult, op1=OP.add)
                                nc.vector.tensor_scalar(
                                    k_sb[:, o, CH * r:CH * (r + 1)], ps_k[o][:],
                                    RWS, bk_sb[:, o, :], op0=OP.mult, op1=OP.add)
                            for s in range(S_SUB):
                                nc.vector.scalar_tensor_tensor(
                                    v_sb[:, r * S_SUB + s, :], ps_v[s][:], RWS,
                                    bv_sb[:], op0=OP.mult, op1=OP.add)

                    # ========= Phase B2: attention per (head, batch) =========
                    WPC = NCT * NB * 2 // 8
                    wp_pieces = iter(range(8))
                    with tc.tile_pool(name="attn_e", bufs=4) as ep, \
                         tc.tile_pool(name="attn_acc", bufs=2) as accp, \
                         tc.tile_pool(name="attn_small", bufs=3) as asml, \
                         tc.tile_pool(name="vals", bufs=3) as valsp, \
                         tc.tile_pool(name="psS", bufs=3, space="PSUM") as psS, \
                         tc.tile_pool(name="psAV", bufs=1, space="PSUM") as psAV, \
                         tc.tile_pool(name="psDen", bufs=1, space="PSUM") as psDen:
                        for h in range(H_PER_CORE):
                            for bb in range(B):
                                for j in range(NQS):
                                    # stream the proj weights behind the n1c
                                    # loads, spread so no critical transfer is
                                    # ever stuck behind a long one
                                    pc = next(wp_pieces, None)
                                    if pc is not None:
                                        nc.sync.dma_start(
                                            wp_sb[:, WPC * pc:WPC * (pc + 1), :],
                                            wproj8[:, WPC * pc:WPC * (pc + 1), :])
                                    ni = 4 * (j + 1) if QS == 512 else T // 128
                                    ps_av = psAV.tile([128, QS], F32, tag="av",
                                                      name="ps_av")
                                    ps_den = psDen.tile([128, QS], F32, tag="den",
                                                        name="ps_den")
                                    e_acc = accp.tile([128, QS], BF16, tag="eacc",
                                                      name="e_acc")
                                    qtok = bb * T + j * QS
                                    for u in range(ni // 2):
                                        # paired score tiles share one Exp call
                                        # over [128, 1024] (amortizes Act setup)
                                        ps_s2 = psS.tile([128, 2, QS], F32,
                                                         tag="s2", name="ps_s2")
                                        for hf in range(2):
                                            i = 2 * u + hf
                                            ktok = bb * T + i * 128
                                            nc.tensor.matmul(
                                                ps_s2[:, hf, :],
                                                k_sb[:, h, ktok:ktok + 128],
                                                q_sb[:, h, qtok:qtok + QS],
                                                start=True, stop=True)
                                        e2 = ep.tile([128, 2, QS], BF16, tag="e2",
                                                     name="e2")
                                        nc.scalar.activation(e2[:], ps_s2[:],
                                                             AF.Exp, bias=0.0,
                                                             scale=SCALE)
                                        d0 = 2 * u - (ni - 4)
                                        if d0 >= 0:
                                            nc.vector.tensor_mul(
                                                e2[:], e2[:],
                                                masks_sb[:, d0:d0 + 2, :])
                                        for hf in range(2):
                                            i = 2 * u + hf
                                            # hybrid denominator: diagonal tiles
                                            # accumulate on PE, the rest on DVE
                                            if i < ni - 4:
                                                if i == 0:
                                                    nc.vector.tensor_copy(
                                                        e_acc[:], e2[:, hf, :])
                                                else:
                                                    nc.vector.tensor_add(
                                                        e_acc[:], e_acc[:],
                                                        e2[:, hf, :])
                                            else:
                                                nc.tensor.matmul(
                                                    ps_den[:], ones_bf[:],
                                                    e2[:, hf, :],
                                                    start=(i == ni - 4),
                                                    stop=(i == ni - 1 and ni == 4))
                                            tt = (bb * T + i * 128) // 128
                                            nc.tensor.matmul(
                                                ps_av[:],
                                                v_sb[:, tt, 128 * h:128 * (h + 1)],
                                                e2[:, hf, :],
                                                start=(i == 0), stop=(i == ni - 1))
                                    if ni > 4:
                                        nc.tensor.matmul(ps_den[:], ones_bf[:],
                                                         e_acc[:], start=False,
                                                         stop=True)
                                    rec_bc = asml.tile([128, QS], F32, tag="rec",
                                                       name="rec_bc")
                                    nc.vector.reciprocal(rec_bc[:], ps_den[:])
                                    vtile = valsp.tile([128, QS], F8, tag="vt",
                                                       name="vtile")
                                    nc.vector.tensor_mul(vtile[:], ps_av[:],
                                                         rec_bc[:])
                                    ncol0 = bb * T + j * QS
                                    for part in range(max(1, QS // CH)):
                                        jg = (ncol0 + part * CH) // CH
                                        w = min(CH, QS)
                                        nc.sync.dma_start(
                                            a2a_in[h][128 * jg:128 * (jg + 1), :],
                                            vtile[:, part * w:(part + 1) * w])
                            if h == 0:
                                # h=0 values complete at half-time: overlap the
                                # first AllToAll with the h=1 attention pass
                                if collectives:
                                    nc.gpsimd.collective_compute(
                                        "AllToAll", OP.bypass, replica_groups=RG,
                                        ins=[a2a_in[0].opt()],
                                        outs=[a2a_out[0].opt()])
                                else:
                                    nc.sync.dma_start(a2a_out[0][:], a2a_in[0][:])

                wqkvp.release()
                if collectives:
                    nc.gpsimd.collective_compute(
                        "AllToAll", OP.bypass, replica_groups=RG,
                        ins=[a2a_in[1].opt()], outs=[a2a_out[1].opt()])
                else:
                    nc.sync.dma_start(a2a_out[1][:], a2a_in[1][:])

                # ====== Phase C: proj + residual + LN2 stats (own chunk) ======
                with tc.tile_pool(name="vf", bufs=1) as vfp, \
                     tc.tile_pool(name="pdrain", bufs=3) as pdp, \
                     tc.tile_pool(name="lnC_small", bufs=1) as lnCs, \
                     tc.tile_pool(name="psP", bufs=3, space="PSUM") as psP, \
                     tc.tile_pool(name="psP2", bufs=1, space="PSUM") as psP2:
                    vf_sb = vfp.tile([128, NB, 2, CH], F8, tag="vf",
                                     name="vf_sb")
                    for hs in range(2):
                        nc.sync.dma_start(
                            vf_sb[:, :, hs, :],
                            a2a_out[hs][:].rearrange("(r p) t -> p r t", p=128))
                    ps_sum2 = psP2.tile([128, CH], F32, tag="sum2", name="ps_sum2")
                    ps_ssq2 = psP2.tile([128, CH], F32, tag="ssq2", name="ps_ssq2")
                    r1bfp = tc.alloc_tile_pool(name="r1bf", bufs=1)
                    r1_bf = r1bfp.tile([128, NCT, CH], BF16, tag="r1_bf",
                                       name="r1_bf")

                    def ln2_stats(ot):
                        # lag-2 interleave behind the proj loop: r1[ot] is ready
                        # two iterations later, so the PE stats matmuls never
                        # stall on the drain chain
                        if ot % 2 == 0:
                            nc.scalar.activation(r1_bf[:, ot, :], r1_sb[:, ot, :],
                                                 AF.Identity)
                        else:
                            nc.vector.tensor_copy(r1_bf[:, ot, :],
                                                  r1_sb[:, ot, :])
                        sq2 = pdp.tile([128, CH], BF16, tag="sq2", name="sq2")
                        nc.vector.tensor_mul(sq2[:], r1_bf[:, ot, :],
                                             r1_bf[:, ot, :])
                        nc.tensor.matmul(ps_sum2[:], ones_bf[:], r1_bf[:, ot, :],
                                         start=(ot == 0), stop=(ot == NCT - 1))
                        nc.tensor.matmul(ps_ssq2[:], ones_bf[:], sq2[:],
                                         start=(ot == 0), stop=(ot == NCT - 1))

                    for ot in range(NCT):
                        ps_p = psP.tile([128, CH], F32, tag="p", name="ps_p")
                        for b in range(NB):
                            nc.tensor.matmul(
                                ps_p[:],
                                wp_sb[:, (ot * NB + b) * 2:(ot * NB + b) * 2 + 2, :],
                                vf_sb[:, b, :, :],
                                start=(b == 0), stop=(b == NB - 1),
                                perf_mode=PM.DoubleRow)
                        p_t = pdp.tile([128, CH], F32, tag="p_t", name="p_t")
                        nc.scalar.activation(p_t[:], ps_p[:], AF.Identity,
                                             bias=bproj_sb[:, ot, :], scale=RWS)
                        nc.vector.tensor_add(r1_sb[:, ot, :], p_t[:],
                                             x_tile(ot))
                        if ot >= 2:
                            ln2_stats(ot - 2)
                    ln2_stats(NCT - 2)
                    ln2_stats(NCT - 1)
                    rstd2_bf, shift2_bf = _ln_finish(nc, lnCs, ps_sum2, ps_ssq2,
                                                     C, CH, "ln2", lp=nc)
                    for k in range(NCT):
                        if k % 3 == 2:
                            tmp2 = pdp.tile([128, CH], BF16, tag="ap2P",
                                            name="tmp2")
                            nc.gpsimd.tensor_mul(tmp2[:], r1_bf[:, k, :],
                                                 rstd2_bf[:])
                            nc.gpsimd.tensor_add(n2_sb[:, k, :], tmp2[:],
                                                 shift2_bf[:])
                        else:
                            tmp2 = pdp.tile([128, CH], BF16, tag="ap2V",
                                            name="tmp2")
                            nc.vector.tensor_mul(tmp2[:], r1_bf[:, k, :],
                                                 rstd2_bf[:])
                            nc.vector.tensor_add(n2_sb[:, k, :], tmp2[:],
                                                 shift2_bf[:])
                    r1bfp.release()

                projw.release()
                # =============== Phase D: FFN1 (fp8 DR) ===============
                with tc.tile_pool(name="hpool", bufs=1) as hpool, \
                     tc.tile_pool(name="w1", bufs=6) as w1p, \
                     tc.tile_pool(name="psH", bufs=3, space="PSUM") as psH:
                    h_sb = hpool.tile([128, NFT, CH], F8, tag="h_sb",
                                      name="h_sb")
                    for ft in range(NFT):
                        w1t = w1p.tile([128, NB * 2, 128], F8, tag="w1",
                                       name="w1t")
                        nc.sync.dma_start(w1t[:], wf18[ft])
                        ps_h = psH.tile([128, CH], F32, tag="h", name="ps_h")
                        for b in range(NB):
                            nc.tensor.matmul(
                                ps_h[:], w1t[:, 2 * b:2 * b + 2, :],
                                n2_sb[:, 2 * b:2 * b + 2, :],
                                start=(b == 0), stop=(b == NB - 1),
                                perf_mode=PM.DoubleRow)
                        nc.scalar.activation(h_sb[:, ft, :], ps_h[:], GELU,
                                             bias=bf1_sb[:, ft, :], scale=RWS)

                    # ============= Phase D2: FFN2 (fp8 DR) =============
                    with tc.tile_pool(name="w2", bufs=3) as w2p, \
                         tc.tile_pool(name="outp", bufs=3) as outp, \
                         tc.tile_pool(name="psF", bufs=2, space="PSUM") as psF:
                        for ot in range(NCT):
                            w2t = w2p.tile([128, NFB * 2, 128], F8, tag="w2",
                                           name="w2t")
                            nc.sync.dma_start(w2t[:], wf28[ot])
                            ps_f = psF.tile([128, CH], F32, tag="f",
                                            name="ps_f")
                            for g in range(NFB):
                                nc.tensor.matmul(
                                    ps_f[:], w2t[:, 2 * g:2 * g + 2, :],
                                    h_sb[:, 2 * g:2 * g + 2, :],
                                    start=(g == 0), stop=(g == NFB - 1),
                                    perf_mode=PM.DoubleRow)
                            f_t = outp.tile([128, CH], F32, tag="f_t",
                                            name="f_t")
                            nc.scalar.activation(f_t[:], ps_f[:], AF.Identity,
                                                 bias=bf2_sb[:, ot, :],
                                                 scale=RWS)
                            o_t = outp.tile([128, CH], F32, tag="o_t",
                                            name="o_t")
                            nc.vector.tensor_add(o_t[:], f_t[:],
                                                 r1_sb[:, ot, :])
                            nc.sync.dma_start(out[128 * ot:128 * (ot + 1), :],
                                              o_t[:])
                n2pool.release()

    nc.compile()
    return nc


# ----------------------------------------------------------------------------
# Host side
# ----------------------------------------------------------------------------

_NC_CACHE = {}


def _get_nc(T=2048):
    if T not in _NC_CACHE:
        _NC_CACHE[T] = build_decoder(T)
    return _NC_CACHE[T]


def _q8(a):
    """Quantize f32 -> fp8 e4m3 bytes with the x32 pre-scale."""
    return (np.ascontiguousarray(a, np.float32) * WS).astype(
        ml_dtypes.float8_e4m3).view(np.uint8)


def _dr_lhsT_flat(W):
    """[K, M] f32 -> DoubleRow lhsT SBUF layout [128, (M/128 * K/256 * 2), 128]."""
    K, M = W.shape
    a = W.reshape(K // 256, 2, 128, M // 128, 128).transpose(2, 3, 0, 1, 4)
    return np.ascontiguousarray(a.reshape(128, (M // 128) * (K // 256) * 2, 128))


def _dr_lhsT_tiles(W):
    """[K, M] f32 -> per-out-tile DoubleRow layout [M/128, 128, K/256 * 2, 128]."""
    K, M = W.shape
    a = W.reshape(K // 256, 2, 128, M // 128, 128).transpose(3, 2, 0, 1, 4)
    return np.ascontiguousarray(a.reshape(M // 128, 128, (K // 256) * 2, 128))


def _bf16(a):
    return np.ascontiguousarray(a).astype(ml_dtypes.bfloat16).view(np.uint16)


def _prep_inputs(x, Wqkv, bqkv, Wproj, bproj, Wf1, bf1, Wf2, bf2,
                 g1, b1, g2, b2):
    """Fold LN affines, slice heads per core, build per-core in_maps."""
    f32 = np.float32
    x = np.asarray(x, f32)
    Bx, T, Cx = x.shape
    NTOK = Bx * T
    CH = NTOK // N_CORES
    Wqkv = np.asarray(Wqkv, f32)
    bqkv = np.asarray(bqkv, f32)
    g1 = np.asarray(g1, f32); b1 = np.asarray(b1, f32)
    g2 = np.asarray(g2, f32); b2 = np.asarray(b2, f32)
    Wqkv_eff = g1[:, None] * Wqkv
    bqkv_eff = b1 @ Wqkv + bqkv
    Wf1 = np.asarray(Wf1, f32)
    bf1v = np.asarray(bf1, f32)
    Wf1_eff = g2[:, None] * Wf1
    bf1_eff = b2 @ Wf1 + bf1v
    Wproj = np.asarray(Wproj, f32)
    bprojv = np.asarray(bproj, f32)
    Wf2 = np.asarray(Wf2, f32)
    bf2v = np.asarray(bf2, f32)

    xt = np.ascontiguousarray(x.reshape(NTOK, Cx).T)        # [C, NT]

    QS = min(512, T)
    masks = np.zeros((128, 4, QS), f32)
    p = np.arange(128)[:, None]
    fcol = np.arange(QS)[None, :]
    for m in range(4):
        masks[:, m, :] = (p <= fcol - 128 * m).astype(f32)

    shared = {
        "wproj8": _q8(_dr_lhsT_flat(Wproj)),
        "bproj": np.ascontiguousarray(
            bprojv.reshape(Cx // 128, 128).T.reshape(128, Cx // 128, 1)),
        "wf18": _q8(_dr_lhsT_tiles(Wf1_eff)),
        "bf1": np.ascontiguousarray(
            bf1_eff.reshape(F // 128, 128).T.reshape(128, F // 128, 1)),
        "wf28": _q8(_dr_lhsT_tiles(Wf2)),
        "bf2": np.ascontiguousarray(
            bf2v.reshape(Cx // 128, 128).T.reshape(128, Cx // 128, 1)),
        "masks": _bf16(masks),
    }
    in_maps = []
    for c in range(N_CORES):
        h0, h1 = 2 * c, 2 * c + 1
        qcols = np.concatenate([h0 * 384 + np.arange(128),
                                h1 * 384 + np.arange(128)])
        kcols = qcols + 128
        vcols = qcols + 256
        m = dict(shared)
        m["xt"] = np.ascontiguousarray(xt[:, c * CH:(c + 1) * CH])
        m["wq8"] = _q8(_dr_lhsT_flat(Wqkv_eff[:, qcols]))
        m["wk8"] = _q8(_dr_lhsT_flat(Wqkv_eff[:, kcols]))
        # v rhs SBUF layout: [128, K/256 * 2, 256]
        wv = Wqkv_eff[:, vcols].reshape(8, 2, 128, 256).transpose(2, 0, 1, 3)
        m["wv8"] = _q8(np.ascontiguousarray(wv.reshape(128, 16, 256)))
        m["bq"] = np.ascontiguousarray(
            bqkv_eff[qcols].reshape(2, 128).T.reshape(128, 2, 1))
        m["bk"] = np.ascontiguousarray(
            bqkv_eff[kcols].reshape(2, 128).T.reshape(128, 2, 1))
        m["bv_bc"] = np.ascontiguousarray(
            np.broadcast_to(bqkv_eff[vcols][None, :], (128, 256)))
        in_maps.append(m)
    return in_maps, (Bx, T, Cx, CH)


def kernel(x, Wqkv, bqkv, Wproj, bproj, Wf1, bf1, Wf2, bf2,
           g1, b1, g2, b2, _trace=False):
    in_maps, (Bx, T, Cx, CH) = _prep_inputs(
        x, Wqkv, bqkv, Wproj, bproj, Wf1, bf1, Wf2, bf2, g1, b1, g2, b2)
    nc = _get_nc(T)
    res = bass_utils.run_bass_kernel_spmd(
        nc, in_maps, core_ids=list(range(N_CORES)), trace=_trace)
    kernel.last_results = res
    NTOK = Bx * T
    out_t = np.empty((NTOK, Cx), np.float32)
    for c in range(N_CORES):
        out_t[c * CH:(c + 1) * CH, :] = res.results[c]["out"].T
    return out_t.reshape(Bx, T, Cx)
